# revision 1
# baseline (speedup 1.0000x reference)
"""Trainium2 Bass kernel for nn_Attention_13348758356565.

Dense transformer attention block (B=16, N=1024 tokens, DIM=1024, 16 heads x 64)
with axial rotary embeddings, data-parallel over batch across 8 NeuronCores
(2 batches per core). All matmuls on TensorE at full rate: QKV projection in
float32r (~1.5e-4 matmul precision), attention QK^T in float32r with 2-head
row-group packing (K=64), P*V and output projection in bf16. Softmax without
max-subtraction (scores are O(1)); denominators come free from an appended
ones-column in the PV stationary operand; normalization uses a GPSIMD
partition-broadcast of a DVE fast reciprocal.
"""

import os
import sys

sys.path.insert(0, "/opt/trn_rl_repo")

import dataclasses
import numpy as np

import concourse.bacc as bacc
import concourse.mybir as mybir
import concourse.tile as tile
from concourse import bass_utils

F32 = mybir.dt.float32
F32R = mybir.dt.float32r
BF16 = mybir.dt.bfloat16
EXP = mybir.ActivationFunctionType.Exp

B, HF, WF = 16, 32, 32
DIM, NH, HD = 1024, 16, 64
N = HF * WF          # 1024 tokens
NCORES = 8
BPC = B // NCORES    # 2 batches per core
ROT = HD // 2        # 32 rotary dims per head
SCALE = 1.0 / np.sqrt(HD)

last_exec_time_ns = None


def _round13(x):
    """Round fp32 mantissa to 13 bits (safe operand form for f32r matmuls)."""
    xi = np.ascontiguousarray(x, np.float32).view(np.uint32)
    xi = ((xi.astype(np.uint64) + (1 << 9)) >> 10 << 10).astype(np.uint32)
    return xi.view(np.float32)


def _bcast_mid(ap, count):
    """Insert a step-0 (broadcast) middle dim into a [P, C] AP -> [P, count, C]."""
    return dataclasses.replace(ap, ap=[ap.ap[0], [0, count], ap.ap[1]])


def _freq_tables():
    d = HD // 4
    base = (np.linspace(1.0, (HF * WF) / 2.0, d // 2, dtype=np.float64) * np.pi)
    posH = np.linspace(-1.0, 1.0, HF)
    posW = np.linspace(-1.0, 1.0, WF)
    fH = np.repeat(posH[:, None] * base[None, :], 2, axis=-1)   # [H, 16]
    fW = np.repeat(posW[:, None] * base[None, :], 2, axis=-1)   # [W, 16]
    fH = np.broadcast_to(fH[:, None, :], (HF, WF, d))
    fW = np.broadcast_to(fW[None, :, :], (HF, WF, d))
    freqs = np.concatenate([fH, fW], axis=-1).reshape(N, ROT)
    # freqs[:, 2i] == freqs[:, 2i+1]; keep one per pair -> [N, 16]
    half = freqs[:, 0::2].astype(np.float32)
    return np.cos(half).astype(np.float32), np.sin(half).astype(np.float32)


def _build():
    nc = bacc.Bacc("TRN2", target_bir_lowering=False, debug=False)

    xT_d = nc.dram_tensor("xT", [BPC, DIM, N], F32R, kind="ExternalInput")
    wqkvT_d = nc.dram_tensor("wqkvT", [DIM, 3 * DIM], F32R, kind="ExternalInput")
    wprojT_d = nc.dram_tensor("wprojT", [DIM, DIM], BF16, kind="ExternalInput")
    bproj_d = nc.dram_tensor("bproj", [1, DIM], BF16, kind="ExternalInput")
    cosh_d = nc.dram_tensor("cosh", [N, 16], F32, kind="ExternalInput")
    sinh_d = nc.dram_tensor("sinh", [N, 16], F32, kind="ExternalInput")
    ident_d = nc.dram_tensor("ident", [128, 128], F32R, kind="ExternalInput")
    ones_d = nc.dram_tensor("ones", [1, 128], BF16, kind="ExternalInput")
    y_d = nc.dram_tensor("y", [BPC, N, DIM], F32, kind="ExternalOutput")

    NT = N // 128            # 8 token tiles
    ND = DIM // 128          # 8 contraction tiles
    HP = NH // 2             # 8 head pairs

    with tile.TileContext(nc) as tc:
        with (
            tc.tile_pool(name="sb", bufs=1) as sb,
            tc.tile_pool(name="ps", bufs=1, space="PSUM") as ps,
        ):
            # ---- constants ----
            ident = sb.tile([128, 128], F32R, name="ident")
            nc.sync.dma_start(ident[:], ident_d.ap())
            ones_r = sb.tile([1, 128], BF16, name="ones_r")
            nc.sync.dma_start(ones_r[:], ones_d.ap())
            bproj = sb.tile([1, DIM], BF16, name="bproj")
            nc.sync.dma_start(bproj[:], bproj_d.ap())
            cosh = sb.tile([128, NT * 16], F32, name="cosh")
            sinh = sb.tile([128, NT * 16], F32, name="sinh")
            nc.sync.dma_start(
                cosh[:].rearrange("p (t c) -> p t c", c=16),
                cosh_d.ap().rearrange("(t p) c -> p t c", p=128),
            )
            nc.sync.dma_start(
                sinh[:].rearrange("p (t c) -> p t c", c=16),
                sinh_d.ap().rearrange("(t p) c -> p t c", p=128),
            )
            wprojT = [sb.tile([128, DIM], BF16, name=f"wprojT{d}") for d in range(ND)]
            for d in range(ND):
                nc.sync.dma_start(wprojT[d][:], wprojT_d.ap()[d * 128:(d + 1) * 128, :])

            # persistent per-b buffers (tags reused across b)
            qT = [sb.tile([128, N], F32R, name=f"qT{j}", tag=f"qT{j}") for j in range(ND)]
            kT = [sb.tile([128, N], F32R, name=f"kT{j}", tag=f"kT{j}") for j in range(ND)]
            vsb = [sb.tile([128, NH * (HD + 1)], BF16, name=f"v{t}", tag=f"v{t}")
                   for t in range(NT)]

            mul = mybir.AluOpType.mult
            sub = mybir.AluOpType.subtract
            add = mybir.AluOpType.add

            def qkv_chunk(b, xT, tag, j0, width, consume):
                """qkv[:, j0:j0+width] for batch b; consume(t, pq) eats each
                [128, width] psum chunk of token-tile t."""
                nheads = width // HD
                wq = [sb.tile([128, width], F32R, name=f"wq_{tag}_{d}",
                              tag=f"wq{d}", bufs=2) for d in range(ND)]
                for d in range(ND):
                    nc.sync.dma_start(
                        wq[d][:], wqkvT_d.ap()[d * 128:(d + 1) * 128, j0:j0 + width])
                for t in range(NT):
                    pq = ps.tile([128, width], F32, name=f"pq_{tag}_{t}",
                                 tag="mm512", bufs=2)
                    for d in range(ND):
                        nc.tensor.matmul(
                            pq[:], xT[d][:, t * 128:(t + 1) * 128], wq[d][:],
                            start=(d == 0), stop=(d == ND - 1))
                    consume(t, pq)

            def rotary(tag, t, pq, nheads):
                """copy psum out fast (frees the bank), then rotate the first
                32 dims of each head in place on the f32r copy."""
                qn = sb.tile([128, nheads * HD], F32R, name=f"qn_{tag}_{t}",
                             tag="work2k", bufs=9)
                nc.vector.tensor_copy(qn[:], pq[:])   # releases psum slot
                on = qn[:].rearrange("p (h i u) -> p h i u", i=32, u=2)
                ev, od = on[:, :, 0:16, 0], on[:, :, 0:16, 1]
                cb = _bcast_mid(cosh[:, t * 16:(t + 1) * 16], nheads)
                sbb = _bcast_mid(sinh[:, t * 16:(t + 1) * 16], nheads)
                ts4 = []
                for i in range(4):
                    ti = sb.tile([128, nheads, 16], F32, name=f"t{i}_{tag}_{t}",
                                 tag=f"rtmp{i}", bufs=1)
                    ts4.append(ti)
                nc.vector.tensor_tensor(ts4[0][:], ev, cb, mul)
                nc.vector.tensor_tensor(ts4[1][:], od, sbb, mul)
                nc.vector.tensor_tensor(ts4[2][:], od, cb, mul)
                nc.vector.tensor_tensor(ts4[3][:], ev, sbb, mul)
                nc.vector.tensor_tensor(ev, ts4[0][:], ts4[1][:], sub)
                nc.vector.tensor_tensor(od, ts4[2][:], ts4[3][:], add)
                return qn

            def transpose_group(tag, qn4, grp, jt, dst):
                """PE-transpose col jt of 4 natural tiles into dst[:, grp*512:]."""
                tp = ps.tile([128, 512], F32R, name=f"tp_{tag}_{jt}_{grp}",
                             tag="mm512", bufs=2)
                for u in range(4):
                    nc.tensor.transpose(
                        tp[:, u * 128:(u + 1) * 128],
                        qn4[u][:, jt * 128:(jt + 1) * 128], ident[:])
                nc.vector.tensor_copy(dst[:, grp * 512:(grp + 1) * 512], tp[:])

            def attention(b, hp):
                pv = [ps.tile([HD + 1, 512], F32, name=f"pv_b{b}_{hp}_{i}",
                              tag="pv", bufs=4) for i in range(4)]  # A0 A1 B0 B1
                for m in range(NT):
                    for nch in range(2):
                        # row-group packed pair: head A rows 0-63, head B 64-127
                        sts, pts = [], []
                        for half in range(2):
                            r0, r1 = half * 64, half * 64 + 64
                            st = ps.tile([128, 512], F32,
                                         name=f"st_b{b}_{hp}_{m}_{nch}_{half}",
                                         tag="st", bufs=2)
                            nc.tensor.matmul(
                                st[:],
                                kT[hp][r0:r1, m * 128:(m + 1) * 128],
                                qT[hp][r0:r1, nch * 512:(nch + 1) * 512])
                            sts.append(st)
                        for half in range(2):
                            pt = sb.tile([128, 512], BF16,
                                         name=f"pt_b{b}_{hp}_{m}_{nch}_{half}",
                                         tag="work2k", bufs=9)
                            nc.scalar.activation(pt[:], sts[half][:], EXP,
                                                 scale=float(SCALE))
                            pts.append(pt)
                        for half in range(2):
                            h = hp * 2 + half
                            nc.tensor.matmul(
                                pv[half * 2 + nch][:],
                                vsb[m][:, h * (HD + 1):(h + 1) * (HD + 1)],
                                pts[half][:],
                                start=(m == 0), stop=(m == NT - 1))
                # normalize: outT[h] = pv[0:64] * (1/denom); denom = pv row 64
                for half in range(2):
                    h = hp * 2 + half
                    for nch in range(2):
                        p = pv[half * 2 + nch]
                        dr = sb.tile([1, 512], F32, name=f"dr_b{b}_{h}_{nch}",
                                     tag="dr", bufs=1)
                        nc.vector.tensor_copy(dr[:], p[64:65, :])
                        rr = sb.tile([1, 512], F32, name=f"rr_b{b}_{h}_{nch}",
                                     tag="rr", bufs=1)
                        nc.vector.reciprocal_approx_fast(rr[:], dr[:])
                        rb = sb.tile([64, 512], F32, name=f"rb_b{b}_{h}_{nch}",
                                     tag="rb", bufs=1)
                        nc.gpsimd.partition_broadcast(rb[:], rr[:])
                        nc.vector.tensor_tensor(
                            outT[h // 2][(h % 2) * 64:(h % 2) * 64 + 64,
                                         nch * 512:(nch + 1) * 512],
                            p[0:64, :], rb[:], mul)

            for b in range(BPC):
                xT = []
                for d in range(ND):
                    xt = sb.tile([128, N], F32R, name=f"xT_b{b}_{d}", tag=f"xT{d}")
                    nc.sync.dma_start(xt[:], xT_d.ap()[b, d * 128:(d + 1) * 128, :])
                    xT.append(xt)
                for t in range(NT):
                    nc.vector.memset(
                        vsb[t][:].rearrange("p (h c) -> p h c", c=HD + 1)[:, :, HD:],
                        1.0)

                outT = [sb.tile([128, N], BF16, name=f"outT_b{b}_{d}", tag=f"outT{d}",
                                bufs=1) for d in range(ND)]

                # ---- V first (it gates every head's PV) ----
                for jc in range(2):
                    def eat_v(t, pq, jc=jc):
                        h0 = jc * 8
                        nc.vector.tensor_copy(
                            vsb[t][:].rearrange("p (h c) -> p h c", c=HD + 1)
                            [:, h0:h0 + 8, 0:HD],
                            pq[:].rearrange("p (h c) -> p h c", c=HD))
                    qkv_chunk(b, xT, f"v{b}{jc}", 2 * DIM + jc * 512, 512, eat_v)

                # ---- per group: K chunk, Q chunk (4 heads each), then the
                # two head-pairs' attention — keeps PE fed during exp ----
                for g in range(2):  # 8 heads (4 head-pairs) per group
                    for sect, dst_all in ((1, kT), (0, qT)):
                        qn_tiles = []

                        def eat_qk(t, pq, sect=sect, g=g, qn_tiles=qn_tiles):
                            qn_tiles.append(rotary(f"s{sect}b{b}g{g}", t, pq, 8))
                            if t % 4 == 3:
                                grp = t // 4
                                for jt in range(4):
                                    transpose_group(
                                        f"s{sect}b{b}g{g}", qn_tiles[grp * 4:],
                                        grp, jt, dst_all[4 * g + jt])
                        qkv_chunk(b, xT, f"s{sect}b{b}g{g}",
                                  sect * DIM + g * 512, 512, eat_qk)
                    for hp in range(4 * g, 4 * g + 4):
                        attention(b, hp)

                # ================= Phase 3: output projection ==============
                for t in range(NT):
                    for ec in range(2):
                        py = ps.tile([128, 512], F32, name=f"py_b{b}_{t}_{ec}",
                                     tag="mm512", bufs=2)
                        for d in range(ND):
                            nc.tensor.matmul(
                                py[:],
                                outT[d][:, t * 128:(t + 1) * 128],
                                wprojT[d][:, ec * 512:(ec + 1) * 512],
                                start=(d == 0), stop=False,
                            )
                        nc.tensor.matmul(
                            py[:], ones_r[:], bproj[:, ec * 512:(ec + 1) * 512],
                            start=False, stop=True,
                        )
                        ysb = sb.tile([128, 512], F32, name=f"y_b{b}_{t}_{ec}",
                                      tag="work2k", bufs=9)
                        nc.vector.tensor_copy(ysb[:], py[:])
                        nc.sync.dma_start(
                            y_d.ap()[b, t * 128:(t + 1) * 128,
                                     ec * 512:(ec + 1) * 512],
                            ysb[:],
                        )

    nc.compile()
    return nc


_NC_CACHE = None


def kernel(x, w_qkv, w_proj, b_proj):
    global _NC_CACHE, last_exec_time_ns
    x = np.ascontiguousarray(np.asarray(x, np.float32))
    w_qkv = np.asarray(w_qkv, np.float32)
    w_proj = np.asarray(w_proj, np.float32)
    b_proj = np.asarray(b_proj, np.float32)

    if _NC_CACHE is None:
        _NC_CACHE = _build()
    nc = _NC_CACHE

    cos_h, sin_h = _freq_tables()
    wqkvT = _round13(np.ascontiguousarray(w_qkv.T))
    import ml_dtypes
    wprojT16 = np.ascontiguousarray(w_proj.T).astype(ml_dtypes.bfloat16)
    bproj16 = b_proj.reshape(1, DIM).astype(ml_dtypes.bfloat16)
    ones16 = np.ones((1, 128), ml_dtypes.bfloat16)
    ident = np.eye(128, dtype=np.float32)

    in_maps = []
    for c in range(NCORES):
        xs = x[c * BPC:(c + 1) * BPC]                       # [2, N, DIM]
        xT = _round13(np.ascontiguousarray(xs.transpose(0, 2, 1)))
        in_maps.append({
            "xT": xT, "wqkvT": wqkvT, "wprojT": wprojT16,
            "bproj": bproj16, "cosh": cos_h, "sinh": sin_h,
            "ident": ident, "ones": ones16,
        })

    trace = bool(os.environ.get("KERNEL_TRACE"))
    kwargs = {}
    if trace:
        kwargs["trace"] = True
        td = os.environ.get("KERNEL_TRACE_DIR")
        if td:
            kwargs["tmpdir"] = td
    res = bass_utils.run_bass_kernel_spmd(
        nc, in_maps, core_ids=list(range(NCORES)), **kwargs)
    last_exec_time_ns = res.exec_time_ns
    out = np.concatenate([res.results[c]["y"] for c in range(NCORES)], axis=0)
    return np.ascontiguousarray(out.reshape(B, N, DIM).astype(np.float32))


if __name__ == "__main__":
    rng = np.random.default_rng(0)
    xs = rng.standard_normal((B, N, DIM), dtype=np.float32)
    wq = rng.standard_normal((3 * DIM, DIM), dtype=np.float32) / 32
    wp = rng.standard_normal((DIM, DIM), dtype=np.float32) / 32
    bp = np.zeros(DIM, np.float32)
    y = kernel(xs, wq, wp, bp)
    print("y", y.shape, y.dtype, float(np.abs(y).max()))



# revision 3
# speedup vs baseline: 1.3627x; 1.3627x over previous
"""Trainium2 Bass kernel for nn_Attention_13348758356565.

Dense transformer attention block (B=16, N=1024 tokens, DIM=1024, 16 heads x 64)
with axial rotary embeddings, data-parallel over batch across 8 NeuronCores
(2 samples per core). All matmuls bf16 on TensorE at full rate.

Per sample:
- QKV projection x-stationary -> natural [tok, outdim] psum tiles [128, 512]
  (8 heads per half). Rotary applied by DVE directly from psum; V drains into
  [keys, 16*(64+1)] tiles with an interleaved ones column per head (free
  softmax denominator).
- Q/K/attn transposes are dma_start_transpose (xbar) calls: no PE transposes,
  no psum-drain copies. qT/kT layout: [dim%128, (t, dim//128, tok%128)].
- QK^T: kT-slice stationary [64, 128], qT moving [64, 4, 128] -> scores
  [keys, queries] psum; exp on ScalarE -> p bf16 in SBUF.
- P*V: p-chunk stationary [128 keys, 128 queries], moving V[keys, 65] ->
  [queries, 65] psum accumulated over key tiles at full PE rate; denominator
  is column 64 -> normalized by one broadcast tensor_tensor into natural
  attn layout (no partition broadcast needed).
- out-proj: attnT-stationary chunks vs wprojT moving; y written bf16 and
  upcast on host.

Cross-sample emission interleave keeps the PE stream dense during the
exp-gated attention phase so the HAM clock gate stays at 2.4 GHz.
"""

import os
import sys

sys.path.insert(0, "/opt/trn_rl_repo")

import dataclasses
import numpy as np

import concourse.bacc as bacc
import concourse.mybir as mybir
import concourse.tile as tile
from concourse import bass_utils

F32 = mybir.dt.float32
BF16 = mybir.dt.bfloat16
EXP = mybir.ActivationFunctionType.Exp

B, HF, WF = 16, 32, 32
DIM, NH, HD = 1024, 16, 64
N = HF * WF          # 1024 tokens
NCORES = 8
BPC = B // NCORES    # 2 samples per core
NT = N // 128        # 8 token tiles
ND = DIM // 128      # 8 contraction chunks
SCALE = 1.0 / np.sqrt(HD)

mul = mybir.AluOpType.mult
sub = mybir.AluOpType.subtract
add = mybir.AluOpType.add

last_exec_time_ns = None


def _bcast_mid(ap, count):
    """Insert a step-0 (broadcast) middle dim into a [P, C] AP -> [P, count, C]."""
    return dataclasses.replace(ap, ap=[ap.ap[0], [0, count], ap.ap[1]])


def _bcast_last(ap, count):
    """Append a step-0 (broadcast) last dim to an AP -> [..., count]."""
    return dataclasses.replace(ap, ap=list(ap.ap) + [[0, count]])


def _freq_tables():
    d = HD // 4
    base = (np.linspace(1.0, (HF * WF) / 2.0, d // 2, dtype=np.float64) * np.pi)
    posH = np.linspace(-1.0, 1.0, HF)
    posW = np.linspace(-1.0, 1.0, WF)
    fH = np.repeat(posH[:, None] * base[None, :], 2, axis=-1)   # [H, 16]
    fW = np.repeat(posW[:, None] * base[None, :], 2, axis=-1)   # [W, 16]
    fH = np.broadcast_to(fH[:, None, :], (HF, WF, d))
    fW = np.broadcast_to(fW[None, :, :], (HF, WF, d))
    freqs = np.concatenate([fH, fW], axis=-1).reshape(N, HD // 2)
    # freqs[:, 2i] == freqs[:, 2i+1]; keep one per pair -> [N, 16]
    half = freqs[:, 0::2].astype(np.float64)
    # [128, NT, 16]: row p, tile t -> token t*128+p
    cos = np.cos(half).astype(np.float32).reshape(NT, 128, 16).transpose(1, 0, 2)
    sin = np.sin(half).astype(np.float32).reshape(NT, 128, 16).transpose(1, 0, 2)
    return (np.ascontiguousarray(cos.reshape(128, NT * 16)),
            np.ascontiguousarray(sin.reshape(128, NT * 16)))


def _build():
    nc = bacc.Bacc("TRN2", target_bir_lowering=False, debug=False)

    xT_d = nc.dram_tensor("xT", [BPC, DIM, N], BF16, kind="ExternalInput")
    wqkvT_d = nc.dram_tensor("wqkvT", [DIM, 3 * DIM], BF16, kind="ExternalInput")
    wprojT_d = nc.dram_tensor("wprojT", [DIM, DIM], BF16, kind="ExternalInput")
    bproj_d = nc.dram_tensor("bproj", [1, DIM], BF16, kind="ExternalInput")
    ones_d = nc.dram_tensor("ones", [1, 128], BF16, kind="ExternalInput")
    cosn_d = nc.dram_tensor("cosn", [128, NT * 16], F32, kind="ExternalInput")
    sinn_d = nc.dram_tensor("sinn", [128, NT * 16], F32, kind="ExternalInput")
    y_d = nc.dram_tensor("y", [BPC, N, DIM], BF16, kind="ExternalOutput")

    with tile.TileContext(nc) as tc:
        with (
            tc.tile_pool(name="sb", bufs=1) as sb,
            tc.tile_pool(name="ps", bufs=1, space="PSUM") as ps,
        ):
            # ---------------- constants ----------------
            ones_r = sb.tile([1, 128], BF16, name="ones_r")
            nc.sync.dma_start(ones_r[:], ones_d.ap())
            bproj = sb.tile([1, DIM], BF16, name="bproj")
            nc.sync.dma_start(bproj[:], bproj_d.ap())
            cosn = sb.tile([128, NT * 16], F32, name="cosn")
            sinn = sb.tile([128, NT * 16], F32, name="sinn")
            nc.sync.dma_start(cosn[:], cosn_d.ap())
            nc.sync.dma_start(sinn[:], sinn_d.ap())
            wpr = [sb.tile([128, DIM], BF16, name=f"wpr{d}") for d in range(ND)]
            for d in range(ND):
                nc.sync.dma_start(wpr[d][:], wprojT_d.ap()[d * 128:(d + 1) * 128, :])
            # warm the exp table set early (hides the ~2.7us table load)
            expwarm = sb.tile([1, 16], F32, name="expwarm")
            nc.scalar.activation(expwarm[:], cosn[0:1, 0:16], EXP, scale=1.0)

            # ---------------- per-sample inputs ----------------
            def xT_tiles(s):
                ts = []
                for d in range(ND):
                    xt = sb.tile([128, N], BF16, name=f"xT_s{s}_{d}", tag=f"xT{d}")
                    nc.sync.dma_start(xt[:], xT_d.ap()[s, d * 128:(d + 1) * 128, :])
                    ts.append(xt)
                return ts

            wq_cache = {}

            def fetch_w(s, sect, half):
                """stream the [DIM, 512] weight slab for (section, half)."""
                for d in range(ND):
                    wt = sb.tile([128, 512], BF16,
                                 name=f"wq_s{s}_{sect}_{half}_{d}",
                                 tag="wq", bufs=10)
                    nc.sync.dma_start(
                        wt[:],
                        wqkvT_d.ap()[d * 128:(d + 1) * 128,
                                     sect * DIM + half * 512:
                                     sect * DIM + half * 512 + 512])
                    wq_cache[(s, sect, half, d)] = wt

            def proj_psum(s, xT, sect, half, t, tag):
                """[128 tok, 512 outdims] psum tile (heads half*8..half*8+8)."""
                py = ps.tile([128, 512], F32, name=f"py_{tag}", tag="mm512", bufs=3)
                for d in range(ND):
                    nc.tensor.matmul(
                        py[:],
                        xT[d][:, t * 128:(t + 1) * 128],
                        wq_cache[(s, sect, half, d)][:],
                        start=(d == 0), stop=(d == ND - 1))
                return py

            def v_half(s, half, t, xT, vt):
                py = proj_psum(s, xT, 2, half, t, f"v{s}{half}{t}")
                vv = vt[:].rearrange("p (h c) -> p h c", c=HD + 1)
                h0 = half * 8
                nc.vector.memset(vv[:, h0:h0 + 8, HD], 1.0)
                nc.vector.tensor_copy(
                    vv[:, h0:h0 + 8, 0:HD],
                    py[:].rearrange("p (h c) -> p h c", c=HD))
                return vt

            def qk_half(s, sect, half, t, xT, dstT):
                """project half of q (sect=0) or k (sect=1) for token-tile t,
                rotary, transpose into dstT[:, t, half*4:(half+1)*4, :]."""
                py = proj_psum(s, xT, sect, half, t, f"s{sect}_{s}{half}{t}")
                pr = py[:].rearrange("p (h i u) -> p h i u", h=8, i=32, u=2)
                ev, od = pr[:, :, 0:16, 0], pr[:, :, 0:16, 1]
                cb = _bcast_mid(cosn[:, t * 16:(t + 1) * 16], 8)
                sbb = _bcast_mid(sinn[:, t * 16:(t + 1) * 16], 8)
                qn = sb.tile([128, 512], BF16, name=f"qn_{sect}_{s}{half}{t}",
                             tag="qn", bufs=3)
                qr = qn[:].rearrange("p (h i u) -> p h i u", h=8, i=32, u=2)
                t0 = sb.tile([128, 8, 16], BF16, name=f"t0_{sect}_{s}{half}{t}",
                             tag="rt0", bufs=2)
                t1 = sb.tile([128, 8, 16], BF16, name=f"t1_{sect}_{s}{half}{t}",
                             tag="rt1", bufs=2)
                nc.vector.tensor_tensor(t0[:], ev, cb, mul)
                nc.vector.tensor_tensor(t1[:], od, sbb, mul)
                nc.vector.tensor_tensor(qr[:, :, 0:16, 0], t0[:], t1[:], sub)
                t2 = sb.tile([128, 8, 16], BF16, name=f"t2_{sect}_{s}{half}{t}",
                             tag="rt0", bufs=2)
                t3 = sb.tile([128, 8, 16], BF16, name=f"t3_{sect}_{s}{half}{t}",
                             tag="rt1", bufs=2)
                nc.vector.tensor_tensor(t2[:], od, cb, mul)
                nc.vector.tensor_tensor(t3[:], ev, sbb, mul)
                nc.vector.tensor_tensor(qr[:, :, 0:16, 1], t2[:], t3[:], add)
                # pass-through dims 32:64 of each head
                pp = py[:].rearrange("p (h c) -> p h c", c=HD)
                qp = qn[:].rearrange("p (h c) -> p h c", c=HD)
                nc.vector.tensor_copy(qp[:, :, 32:64], pp[:, :, 32:64])
                # transpose [tok, dim-half] -> qT[:, t, half*4:(half+1)*4, :]
                nc.sync.dma_start_transpose(
                    dstT[:].rearrange("p (t c q) -> p t c q", t=NT, c=ND)
                    [:, t, half * 4:(half + 1) * 4, :],
                    qn[:])

            # ---------------- attention ----------------
            def attention_head(s, h, qTt, kTt, vts, at):
                kv = kTt[:].rearrange("p (t c q) -> p t c q", t=NT, c=ND)
                qv = qTt[:].rearrange("p (t c q) -> p t c q", t=NT, c=ND)
                r0 = (h % 2) * 64
                for nch in range(2):
                    p_all = sb.tile([128, 8, 512], BF16, name=f"p_s{s}h{h}n{nch}",
                                    tag="p", bufs=3)
                    for m in range(NT):
                        st = ps.tile([128, 512], F32, tag="st", bufs=3,
                                     name=f"st_{s}_{h}_{nch}_{m}")
                        nc.tensor.matmul(
                            st[:],
                            kv[r0:r0 + 64, m, h // 2],
                            qv[r0:r0 + 64, nch * 4:(nch + 1) * 4, h // 2],
                        )
                        nc.scalar.activation(p_all[:, m, :], st[:],
                                             EXP, scale=float(SCALE))
                    pv = ps.tile([128, 260], F32, name=f"pv_{s}_{h}_{nch}",
                                 tag="pv", bufs=2)
                    for ql in range(4):
                        for m in range(NT):
                            nc.tensor.matmul(
                                pv[:, ql * 65:ql * 65 + 65],
                                p_all[:, m, ql * 128:(ql + 1) * 128],
                                vts[m][:].rearrange("p (h c) -> p h c",
                                                    c=HD + 1)[:, h],
                                start=(m == 0), stop=(m == NT - 1))
                    pvv = pv[:].rearrange("p (q c) -> p q c", c=65)
                    rc = sb.tile([128, 4], F32, name=f"rc_{s}_{h}_{nch}",
                                 tag="rc", bufs=2)
                    nc.vector.reciprocal_approx_fast(rc[:], pvv[:, :, 64])
                    av = at[:].rearrange("p (q h c) -> p q h c", q=NT, h=NH)
                    nc.vector.tensor_tensor(
                        av[:, nch * 4:(nch + 1) * 4, h, :],
                        pvv[:, :, 0:64], _bcast_last(rc[:], HD), mul)

            # ---------------- output projection ----------------
            def proj_out(s, qt, at, atT):
                nc.sync.dma_start_transpose(
                    atT[:].rearrange("p (t c q) -> p t c q", t=NT, c=ND)[:, qt],
                    at[:, qt * 1024:(qt + 1) * 1024])
                atv = atT[:].rearrange("p (t c q) -> p t c q", t=NT, c=ND)
                for half in range(2):
                    py = ps.tile([128, 512], F32, name=f"yp_{s}_{qt}_{half}",
                                 tag="mm512", bufs=3)
                    for d in range(ND):
                        nc.tensor.matmul(
                            py[:],
                            atv[:, qt, d],
                            wpr[d][:, half * 512:(half + 1) * 512],
                            start=(d == 0), stop=False)
                    nc.tensor.matmul(
                        py[:],
                        ones_r[:], bproj[:, half * 512:(half + 1) * 512],
                        start=False, stop=True)
                    ysb = sb.tile([128, 512], BF16, name=f"y_{s}_{qt}_{half}",
                                  tag="ysb", bufs=2)
                    nc.vector.tensor_copy(ysb[:], py[:])
                    nc.sync.dma_start(
                        y_d.ap()[s, qt * 128:(qt + 1) * 128,
                                 half * 512:(half + 1) * 512],
                        ysb[:])

            # ================= emission schedule =================
            qT = [sb.tile([128, NT * ND * 128], BF16, name=f"qT_s{s}", tag="qT",
                          bufs=2) for s in range(BPC)]
            kT = [sb.tile([128, NT * ND * 128], BF16, name=f"kT_s{s}", tag="kT",
                          bufs=1) for s in range(BPC)]
            attn = [sb.tile([128, NT * 1024], BF16, name=f"attn_s{s}", tag="attn",
                            bufs=2) for s in range(BPC)]
            attnT = [sb.tile([128, NT * ND * 128], BF16, name=f"attnT_s{s}",
                             tag="attnT", bufs=1) for s in range(BPC)]
            vsb = [[sb.tile([128, NH * (HD + 1)], BF16, name=f"v_s{s}_{t}",
                            tag=f"v{t}", bufs=2) for t in range(NT)]
                   for s in range(BPC)]

            def proj_section(s, sect, xT):
                for half in range(2):
                    fetch_w(s, sect, half)
                    for t in range(NT):
                        if sect == 2:
                            v_half(s, half, t, xT, vsb[s][t])
                        else:
                            qk_half(s, sect, half, t, xT,
                                    qT[s] if sect == 0 else kT[s])

            # ---- phase P(s0): full projection of sample 0 ----
            xT0 = xT_tiles(0)
            proj_section(0, 2, xT0)   # V first
            proj_section(0, 0, xT0)   # Q
            proj_section(0, 1, xT0)   # K

            # ---- phase A(s0) interleaved with V+Q projection of s1 ----
            xT1 = xT_tiles(1)
            s1_filler = []
            for half in range(2):
                s1_filler.append(("w", (1, 2, half)))
                for t in range(NT):
                    s1_filler.append(("v", (half, t)))
            for half in range(2):
                s1_filler.append(("w", (1, 0, half)))
                for t in range(NT):
                    s1_filler.append(("q", (half, t)))
            # 36 filler items over 16 heads
            fi = 0
            for h in range(NH):
                attention_head(0, h, qT[0], kT[0], vsb[0], attn[0])
                take = (len(s1_filler) * (h + 1)) // NH - fi
                for _ in range(take):
                    kind, args = s1_filler[fi]; fi += 1
                    if kind == "w":
                        fetch_w(*args)
                    elif kind == "v":
                        v_half(1, args[0], args[1], xT1, vsb[1][args[1]])
                    else:
                        qk_half(1, 0, args[0], args[1], xT1, qT[1])

            # ---- K projection of s1 ----
            proj_section(1, 1, xT1)

            # ---- phase A(s1) interleaved with out-proj of s0 ----
            for h in range(NH):
                attention_head(1, h, qT[1], kT[1], vsb[1], attn[1])
                if h % 2 == 1:
                    proj_out(0, h // 2, attn[0], attnT[0])

            # ---- out-proj of s1 ----
            for qt in range(NT):
                proj_out(1, qt, attn[1], attnT[1])

    nc.compile()
    return nc


_NC_CACHE = None


def kernel(x, w_qkv, w_proj, b_proj):
    global _NC_CACHE, last_exec_time_ns
    import ml_dtypes

    x = np.asarray(x, np.float32)
    w_qkv = np.asarray(w_qkv, np.float32)
    w_proj = np.asarray(w_proj, np.float32)
    b_proj = np.asarray(b_proj, np.float32)

    if _NC_CACHE is None:
        _NC_CACHE = _build()
    nc = _NC_CACHE

    cosn, sinn = _freq_tables()
    wqkvT = np.ascontiguousarray(w_qkv.T).astype(ml_dtypes.bfloat16)
    wprojT = np.ascontiguousarray(w_proj.T).astype(ml_dtypes.bfloat16)
    bproj16 = b_proj.reshape(1, DIM).astype(ml_dtypes.bfloat16)
    ones16 = np.ones((1, 128), ml_dtypes.bfloat16)

    in_maps = []
    for c in range(NCORES):
        xs = x[c * BPC:(c + 1) * BPC]                       # [2, N, DIM]
        xT = np.ascontiguousarray(xs.transpose(0, 2, 1)).astype(ml_dtypes.bfloat16)
        in_maps.append({
            "xT": xT, "wqkvT": wqkvT, "wprojT": wprojT,
            "bproj": bproj16, "ones": ones16, "cosn": cosn, "sinn": sinn,
        })

    trace = bool(os.environ.get("KERNEL_TRACE"))
    kwargs = {}
    if trace:
        kwargs["trace"] = True
        td = os.environ.get("KERNEL_TRACE_DIR")
        if td:
            kwargs["tmpdir"] = td
    res = bass_utils.run_bass_kernel_spmd(
        nc, in_maps, core_ids=list(range(NCORES)), **kwargs)
    last_exec_time_ns = res.exec_time_ns
    out = np.concatenate([np.asarray(res.results[c]["y"]) for c in range(NCORES)],
                         axis=0)
    return np.ascontiguousarray(out.reshape(B, N, DIM).astype(np.float32))


if __name__ == "__main__":
    rng = np.random.default_rng(0)
    xs = rng.standard_normal((B, N, DIM), dtype=np.float32)
    wq = rng.standard_normal((3 * DIM, DIM), dtype=np.float32) / 32
    wp = rng.standard_normal((DIM, DIM), dtype=np.float32) / 32
    bp = np.zeros(DIM, np.float32)
    y = kernel(xs, wq, wp, bp)
    print("y", y.shape, y.dtype, float(np.abs(y).max()))


# revision 5
# speedup vs baseline: 1.7502x; 1.2844x over previous
"""Trainium2 Bass kernel for nn_Attention_13348758356565.

Dense transformer attention block (B=16, N=1024 tokens, DIM=1024, 16 heads x 64)
with axial rotary embeddings, data-parallel over batch across 8 NeuronCores
(2 samples per core). All matmuls bf16 on TensorE at full rate.

Per sample:
- QKV projection x-stationary -> natural [tok, outdim] psum tiles [128, 512]
  (8 heads per half). Rotary applied by DVE directly from psum; V drains into
  [keys, 16*(64+1)] tiles with an interleaved ones column per head (free
  softmax denominator).
- Q/K/attn transposes are dma_start_transpose (xbar) calls: no PE transposes,
  no psum-drain copies. qT/kT layout: [dim%128, (t, dim//128, tok%128)].
- QK^T: kT-slice stationary [64, 128], qT moving [64, 4, 128] -> scores
  [keys, queries] psum; exp on ScalarE -> p bf16 in SBUF.
- P*V: p-chunk stationary [128 keys, 128 queries], moving V[keys, 65] ->
  [queries, 65] psum accumulated over key tiles at full PE rate; denominator
  is column 64 -> normalized by one broadcast tensor_tensor into natural
  attn layout (no partition broadcast needed).
- out-proj: attnT-stationary chunks vs wprojT moving; y written bf16 and
  upcast on host.

Cross-sample emission interleave keeps the PE stream dense during the
exp-gated attention phase so the HAM clock gate stays at 2.4 GHz.
"""

import os
import sys

sys.path.insert(0, "/opt/trn_rl_repo")

import dataclasses
import numpy as np

import concourse.bacc as bacc
import concourse.mybir as mybir
import concourse.tile as tile
from concourse import bass_utils

F32 = mybir.dt.float32
BF16 = mybir.dt.bfloat16
EXP = mybir.ActivationFunctionType.Exp

B, HF, WF = 16, 32, 32
DIM, NH, HD = 1024, 16, 64
N = HF * WF          # 1024 tokens
NCORES = 8
BPC = B // NCORES    # 2 samples per core
NT = N // 128        # 8 token tiles
ND = DIM // 128      # 8 contraction chunks
SCALE = 1.0 / np.sqrt(HD)

mul = mybir.AluOpType.mult
sub = mybir.AluOpType.subtract
add = mybir.AluOpType.add

last_exec_time_ns = None


def _bcast_mid(ap, count):
    """Insert a step-0 (broadcast) middle dim into a [P, C] AP -> [P, count, C]."""
    return dataclasses.replace(ap, ap=[ap.ap[0], [0, count], ap.ap[1]])


def _bcast_last(ap, count):
    """Append a step-0 (broadcast) last dim to an AP -> [..., count]."""
    return dataclasses.replace(ap, ap=list(ap.ap) + [[0, count]])


def _freq_tables():
    d = HD // 4
    base = (np.linspace(1.0, (HF * WF) / 2.0, d // 2, dtype=np.float64) * np.pi)
    posH = np.linspace(-1.0, 1.0, HF)
    posW = np.linspace(-1.0, 1.0, WF)
    fH = np.repeat(posH[:, None] * base[None, :], 2, axis=-1)   # [H, 16]
    fW = np.repeat(posW[:, None] * base[None, :], 2, axis=-1)   # [W, 16]
    fH = np.broadcast_to(fH[:, None, :], (HF, WF, d))
    fW = np.broadcast_to(fW[None, :, :], (HF, WF, d))
    freqs = np.concatenate([fH, fW], axis=-1).reshape(N, HD // 2)
    # freqs[:, 2i] == freqs[:, 2i+1]; keep one per pair -> [N, 16]
    half = freqs[:, 0::2].astype(np.float64)
    # [128, NT, 16]: row p, tile t -> token t*128+p
    cos = np.cos(half).astype(np.float32).reshape(NT, 128, 16).transpose(1, 0, 2)
    sin = np.sin(half).astype(np.float32).reshape(NT, 128, 16).transpose(1, 0, 2)
    return (np.ascontiguousarray(cos.reshape(128, NT * 16)),
            np.ascontiguousarray(sin.reshape(128, NT * 16)))


def _build():
    nc = bacc.Bacc("TRN2", target_bir_lowering=False, debug=False)

    xT_d = nc.dram_tensor("xT", [BPC, DIM, N], BF16, kind="ExternalInput")
    wqkvT_d = nc.dram_tensor("wqkvT", [DIM, 3 * DIM], BF16, kind="ExternalInput")
    wprojT_d = nc.dram_tensor("wprojT", [DIM, DIM], BF16, kind="ExternalInput")
    bproj_d = nc.dram_tensor("bproj", [1, DIM], BF16, kind="ExternalInput")
    ones_d = nc.dram_tensor("ones", [1, 128], BF16, kind="ExternalInput")
    cosn_d = nc.dram_tensor("cosn", [128, NT * 16], BF16, kind="ExternalInput")
    sinn_d = nc.dram_tensor("sinn", [128, NT * 16], BF16, kind="ExternalInput")
    y_d = nc.dram_tensor("y", [BPC, N, DIM], BF16, kind="ExternalOutput")

    with tile.TileContext(nc) as tc:
        with (
            tc.tile_pool(name="sb", bufs=1) as sb,
            tc.tile_pool(name="ps", bufs=1, space="PSUM") as ps,
        ):
            # ---------------- constants ----------------
            ones_r = sb.tile([1, 128], BF16, name="ones_r")
            nc.sync.dma_start(ones_r[:], ones_d.ap())
            bproj = sb.tile([1, DIM], BF16, name="bproj")
            nc.sync.dma_start(bproj[:], bproj_d.ap())
            cosn = sb.tile([128, NT * 16], BF16, name="cosn")
            sinn = sb.tile([128, NT * 16], BF16, name="sinn")
            nc.sync.dma_start(cosn[:], cosn_d.ap())
            nc.sync.dma_start(sinn[:], sinn_d.ap())
            wpr = [sb.tile([128, DIM], BF16, name=f"wpr{d}") for d in range(ND)]
            for d in range(ND):
                nc.sync.dma_start(wpr[d][:], wprojT_d.ap()[d * 128:(d + 1) * 128, :])
            # warm the exp table set early (hides the ~2.7us table load)
            expwarm = sb.tile([1, 16], F32, name="expwarm")
            nc.scalar.activation(expwarm[:], cosn[0:1, 0:16], EXP, scale=1.0)

            # ---------------- per-sample inputs ----------------
            def xT_tiles(s):
                ts = []
                for d in range(ND):
                    xt = sb.tile([128, N], BF16, name=f"xT_s{s}_{d}", tag=f"xT{d}")
                    nc.sync.dma_start(xt[:], xT_d.ap()[s, d * 128:(d + 1) * 128, :])
                    ts.append(xt)
                return ts

            wq_cache = {}

            def fetch_w(s, sect, half):
                """stream the [DIM, 512] weight slab for (section, half)."""
                for d in range(ND):
                    wt = sb.tile([128, 512], BF16,
                                 name=f"wq_s{s}_{sect}_{half}_{d}",
                                 tag="wq", bufs=11)
                    nc.sync.dma_start(
                        wt[:],
                        wqkvT_d.ap()[d * 128:(d + 1) * 128,
                                     sect * DIM + half * 512:
                                     sect * DIM + half * 512 + 512])
                    wq_cache[(s, sect, half, d)] = wt

            def proj_psum(s, xT, sect, half, t, tag):
                """[128 tok, 512 outdims] psum tile (heads half*8..half*8+8)."""
                py = ps.tile([128, 512], F32, name=f"py_{tag}", tag="mm512", bufs=2)
                for d in range(ND):
                    nc.tensor.matmul(
                        py[:],
                        xT[d][:, t * 128:(t + 1) * 128],
                        wq_cache[(s, sect, half, d)][:],
                        start=(d == 0), stop=(d == ND - 1))
                return py

            def v_half(s, half, t, xT, vt):
                py = proj_psum(s, xT, 2, half, t, f"v{s}{half}{t}")
                vv = vt[:].rearrange("p (h c) -> p h c", c=HD + 1)
                h0 = half * 8
                nc.vector.memset(vv[:, h0:h0 + 8, HD], 1.0)
                nc.vector.tensor_copy(
                    vv[:, h0:h0 + 8, 0:HD],
                    py[:].rearrange("p (h c) -> p h c", c=HD))
                return vt

            def qk_half(s, sect, half, t, xT, dstT):
                """project half of q (sect=0) or k (sect=1) for token-tile t,
                rotary, transpose into dstT[:, t, half*4:(half+1)*4, :]."""
                py = proj_psum(s, xT, sect, half, t, f"s{sect}_{s}{half}{t}")
                pr = py[:].rearrange("p (h i u) -> p h i u", h=8, i=32, u=2)
                ev, od = pr[:, :, 0:16, 0], pr[:, :, 0:16, 1]
                cb = _bcast_mid(cosn[:, t * 16:(t + 1) * 16], 8)
                sbb = _bcast_mid(sinn[:, t * 16:(t + 1) * 16], 8)
                qn = sb.tile([128, 512], BF16, name=f"qn_{sect}_{s}{half}{t}",
                             tag="qn", bufs=2)
                qr = qn[:].rearrange("p (h i u) -> p h i u", h=8, i=32, u=2)
                t0 = sb.tile([128, 8, 16], BF16, name=f"t0_{sect}_{s}{half}{t}",
                             tag="rt0", bufs=2)
                t1 = sb.tile([128, 8, 16], BF16, name=f"t1_{sect}_{s}{half}{t}",
                             tag="rt1", bufs=2)
                nc.vector.tensor_tensor(t0[:], ev, cb, mul)
                nc.vector.tensor_tensor(t1[:], od, sbb, mul)
                nc.vector.tensor_tensor(qr[:, :, 0:16, 0], t0[:], t1[:], sub)
                t2 = sb.tile([128, 8, 16], BF16, name=f"t2_{sect}_{s}{half}{t}",
                             tag="rt0", bufs=2)
                t3 = sb.tile([128, 8, 16], BF16, name=f"t3_{sect}_{s}{half}{t}",
                             tag="rt1", bufs=2)
                nc.vector.tensor_tensor(t2[:], od, cb, mul)
                nc.vector.tensor_tensor(t3[:], ev, sbb, mul)
                nc.vector.tensor_tensor(qr[:, :, 0:16, 1], t2[:], t3[:], add)
                # pass-through dims 32:64 of each head
                pp = py[:].rearrange("p (h c) -> p h c", c=HD)
                qp = qn[:].rearrange("p (h c) -> p h c", c=HD)
                nc.vector.tensor_copy(qp[:, :, 32:64], pp[:, :, 32:64])
                # transpose [tok, dim-half] -> qT[:, t, half*4:(half+1)*4, :]
                nc.sync.dma_start_transpose(
                    dstT[:].rearrange("p (t c q) -> p t c q", t=NT, c=ND)
                    [:, t, half * 4:(half + 1) * 4, :],
                    qn[:])

            # ---------------- attention ----------------
            def attention_head(s, h, qTt, kTt, vts, at):
                kv = kTt[:].rearrange("p (t c q) -> p t c q", t=NT, c=ND)
                qv = qTt[:].rearrange("p (t c q) -> p t c q", t=NT, c=ND)
                r0 = (h % 2) * 64
                for nch in range(2):
                    p_q = [sb.tile([128, 4, 512], BF16,
                                   name=f"p_s{s}h{h}n{nch}q{i}", tag="p", bufs=3)
                           for i in range(2)]
                    for mp in range(NT // 2):
                        st = ps.tile([128, 1024], F32, tag="st", bufs=2,
                                     name=f"st_{s}_{h}_{nch}_{mp}")
                        for u in range(2):
                            nc.tensor.matmul(
                                st[:, u * 512:(u + 1) * 512],
                                kv[r0:r0 + 64, mp * 2 + u, h // 2],
                                qv[r0:r0 + 64, nch * 4:(nch + 1) * 4, h // 2],
                            )
                        nc.scalar.activation(
                            p_q[mp // 2][:, (mp % 2) * 2:(mp % 2) * 2 + 2, :],
                            st[:], EXP, scale=float(SCALE))
                    pv = ps.tile([128, 260], F32, name=f"pv_{s}_{h}_{nch}",
                                 tag="pv", bufs=2)
                    for ql in range(4):
                        for m in range(NT):
                            nc.tensor.matmul(
                                pv[:, ql * 65:ql * 65 + 65],
                                p_q[m // 4][:, m % 4, ql * 128:(ql + 1) * 128],
                                vts[m][:].rearrange("p (h c) -> p h c",
                                                    c=HD + 1)[:, h],
                                start=(m == 0), stop=(m == NT - 1))
                    pvv = pv[:].rearrange("p (q c) -> p q c", c=65)
                    rc = sb.tile([128, 4], F32, name=f"rc_{s}_{h}_{nch}",
                                 tag="rc", bufs=2)
                    nc.vector.reciprocal_approx_fast(rc[:], pvv[:, :, 64])
                    av = at[:].rearrange("p (q h c) -> p q h c", q=NT, h=NH)
                    nc.vector.tensor_tensor(
                        av[:, nch * 4:(nch + 1) * 4, h, :],
                        pvv[:, :, 0:64], _bcast_last(rc[:], HD), mul)

            # ---------------- output projection ----------------
            def proj_out(s, qt, at, atT):
                nc.sync.dma_start_transpose(
                    atT[:].rearrange("p (t c q) -> p t c q", t=NT, c=ND)[:, qt],
                    at[:, qt * 1024:(qt + 1) * 1024])
                atv = atT[:].rearrange("p (t c q) -> p t c q", t=NT, c=ND)
                for half in range(2):
                    py = ps.tile([128, 512], F32, name=f"yp_{s}_{qt}_{half}",
                                 tag="mm512", bufs=2)
                    for d in range(ND):
                        nc.tensor.matmul(
                            py[:],
                            atv[:, qt, d],
                            wpr[d][:, half * 512:(half + 1) * 512],
                            start=(d == 0), stop=False)
                    nc.tensor.matmul(
                        py[:],
                        ones_r[:], bproj[:, half * 512:(half + 1) * 512],
                        start=False, stop=True)
                    ysb = sb.tile([128, 512], BF16, name=f"y_{s}_{qt}_{half}",
                                  tag="ysb", bufs=2)
                    nc.vector.tensor_copy(ysb[:], py[:])
                    nc.sync.dma_start(
                        y_d.ap()[s, qt * 128:(qt + 1) * 128,
                                 half * 512:(half + 1) * 512],
                        ysb[:])

            # ================= emission schedule =================
            qT = [sb.tile([128, NT * ND * 128], BF16, name=f"qT_s{s}", tag="qT",
                          bufs=2) for s in range(BPC)]
            kT = [sb.tile([128, NT * ND * 128], BF16, name=f"kT_s{s}", tag="kT",
                          bufs=2) for s in range(BPC)]
            attn = [sb.tile([128, NT * 1024], BF16, name=f"attn_s{s}", tag="attn",
                            bufs=2) for s in range(BPC)]
            attnT = [sb.tile([128, NT * ND * 128], BF16, name=f"attnT_s{s}",
                             tag="attnT", bufs=1) for s in range(BPC)]
            vsb = [[sb.tile([128, NH * (HD + 1)], BF16, name=f"v_s{s}_{t}",
                            tag=f"v{t}", bufs=2) for t in range(NT)]
                   for s in range(BPC)]

            def slab_items(s, sect, half, xT):
                """fetch + the 8 per-t work items for one weight slab."""
                items = [("w", (s, sect, half))]
                for t in range(NT):
                    if sect == 2:
                        items.append(("v", (s, half, t, xT)))
                    else:
                        items.append(("qk", (s, sect, half, t, xT)))
                return items

            def run_item(it):
                kind, args = it
                if kind == "w":
                    fetch_w(*args)
                elif kind == "v":
                    s_, half, t, xT = args
                    v_half(s_, half, t, xT, vsb[s_][t])
                else:
                    s_, sect, half, t, xT = args
                    qk_half(s_, sect, half, t, xT,
                            qT[s_] if sect == 0 else kT[s_])

            def run_slabs(slabs):
                """emit slab work with fetches hoisted 2 slabs ahead."""
                items = []
                for i, (s_, sect, xT) in enumerate(slabs):
                    for half in range(2):
                        items.append(slab_items(s_, sect, half, xT))
                # reorder: fetch of slab i+2 goes before slab i's t-work
                out = []
                fetched = 0
                for i in range(len(items)):
                    while fetched <= min(i + 2, len(items) - 1):
                        out.append(items[fetched][0]); fetched += 1
                    out.extend(items[i][1:])
                return out

            # ---- phase P(s0): full projection of sample 0 (V, K, Q) ----
            xT0 = xT_tiles(0)
            for it in run_slabs([(0, 2, xT0), (0, 1, xT0), (0, 0, xT0)]):
                run_item(it)

            # ---- phase A(s0) interleaved with full projection of s1 ----
            xT1 = xT_tiles(1)
            s1_filler = run_slabs([(1, 2, xT1), (1, 1, xT1), (1, 0, xT1)])
            fi = 0
            for h in range(NH):
                attention_head(0, h, qT[0], kT[0], vsb[0], attn[0])
                take = (len(s1_filler) * (h + 1)) // NH - fi
                for _ in range(take):
                    run_item(s1_filler[fi]); fi += 1

            # ---- phase A(s1) interleaved with out-proj of s0 ----
            for h in range(NH):
                attention_head(1, h, qT[1], kT[1], vsb[1], attn[1])
                if h % 2 == 1:
                    proj_out(0, h // 2, attn[0], attnT[0])

            # ---- out-proj of s1 ----
            for qt in range(NT):
                proj_out(1, qt, attn[1], attnT[1])

    nc.compile()
    return nc


_NC_CACHE = None


def kernel(x, w_qkv, w_proj, b_proj):
    global _NC_CACHE, last_exec_time_ns
    import ml_dtypes

    x = np.asarray(x, np.float32)
    w_qkv = np.asarray(w_qkv, np.float32)
    w_proj = np.asarray(w_proj, np.float32)
    b_proj = np.asarray(b_proj, np.float32)

    if _NC_CACHE is None:
        _NC_CACHE = _build()
    nc = _NC_CACHE

    cosn, sinn = _freq_tables()
    cosn = cosn.astype(ml_dtypes.bfloat16)
    sinn = sinn.astype(ml_dtypes.bfloat16)
    wqkvT = np.ascontiguousarray(w_qkv.T).astype(ml_dtypes.bfloat16)
    wprojT = np.ascontiguousarray(w_proj.T).astype(ml_dtypes.bfloat16)
    bproj16 = b_proj.reshape(1, DIM).astype(ml_dtypes.bfloat16)
    ones16 = np.ones((1, 128), ml_dtypes.bfloat16)

    in_maps = []
    for c in range(NCORES):
        xs = x[c * BPC:(c + 1) * BPC]                       # [2, N, DIM]
        xT = np.ascontiguousarray(xs.transpose(0, 2, 1)).astype(ml_dtypes.bfloat16)
        in_maps.append({
            "xT": xT, "wqkvT": wqkvT, "wprojT": wprojT,
            "bproj": bproj16, "ones": ones16, "cosn": cosn, "sinn": sinn,
        })

    trace = bool(os.environ.get("KERNEL_TRACE"))
    kwargs = {}
    if trace:
        kwargs["trace"] = True
        td = os.environ.get("KERNEL_TRACE_DIR")
        if td:
            kwargs["tmpdir"] = td
    res = bass_utils.run_bass_kernel_spmd(
        nc, in_maps, core_ids=list(range(NCORES)), **kwargs)
    last_exec_time_ns = res.exec_time_ns
    out = np.concatenate([np.asarray(res.results[c]["y"]) for c in range(NCORES)],
                         axis=0)
    return np.ascontiguousarray(out.reshape(B, N, DIM).astype(np.float32))


if __name__ == "__main__":
    rng = np.random.default_rng(0)
    xs = rng.standard_normal((B, N, DIM), dtype=np.float32)
    wq = rng.standard_normal((3 * DIM, DIM), dtype=np.float32) / 32
    wp = rng.standard_normal((DIM, DIM), dtype=np.float32) / 32
    bp = np.zeros(DIM, np.float32)
    y = kernel(xs, wq, wp, bp)
    print("y", y.shape, y.dtype, float(np.abs(y).max()))


# revision 8
# speedup vs baseline: 1.7821x; 1.0182x over previous
"""Trainium2 Bass kernel for nn_Attention_13348758356565.

Dense transformer attention block (B=16, N=1024 tokens, DIM=1024, 16 heads x 64)
with axial rotary embeddings, data-parallel over batch across 8 NeuronCores
(2 samples per core). All matmuls bf16 on TensorE at full rate.

Per sample:
- QKV projection x-stationary -> natural [tok, outdim] psum tiles [128, 512]
  (8 heads per half). Rotary applied by DVE directly from psum; V drains into
  [keys, 16*(64+1)] tiles with an interleaved ones column per head (free
  softmax denominator).
- Q/K/attn transposes are dma_start_transpose (xbar) calls: no PE transposes,
  no psum-drain copies. qT/kT layout: [dim%128, (t, dim//128, tok%128)].
- QK^T: kT-slice stationary [64, 128], qT moving [64, 4, 128] -> scores
  [keys, queries] psum; exp on ScalarE -> p bf16 in SBUF.
- P*V: p-chunk stationary [128 keys, 128 queries], moving V[keys, 65] ->
  [queries, 65] psum accumulated over key tiles at full PE rate; denominator
  is column 64 -> normalized by one broadcast tensor_tensor into natural
  attn layout (no partition broadcast needed).
- out-proj: attnT-stationary chunks vs wprojT moving; y written bf16 and
  upcast on host.

Cross-sample emission interleave keeps the PE stream dense during the
exp-gated attention phase so the HAM clock gate stays at 2.4 GHz.
"""

import os
import sys

sys.path.insert(0, "/opt/trn_rl_repo")

import dataclasses
import numpy as np

import concourse.bacc as bacc
import concourse.mybir as mybir
import concourse.tile as tile
from concourse import bass_utils

F32 = mybir.dt.float32
BF16 = mybir.dt.bfloat16
EXP = mybir.ActivationFunctionType.Exp

B, HF, WF = 16, 32, 32
DIM, NH, HD = 1024, 16, 64
N = HF * WF          # 1024 tokens
NCORES = 8
BPC = B // NCORES    # 2 samples per core
NT = N // 128        # 8 token tiles
ND = DIM // 128      # 8 contraction chunks
SCALE = 1.0 / np.sqrt(HD)

mul = mybir.AluOpType.mult
sub = mybir.AluOpType.subtract
add = mybir.AluOpType.add

last_exec_time_ns = None


def _bcast_mid(ap, count):
    """Insert a step-0 (broadcast) middle dim into a [P, C] AP -> [P, count, C]."""
    return dataclasses.replace(ap, ap=[ap.ap[0], [0, count], ap.ap[1]])


def _bcast_last(ap, count):
    """Append a step-0 (broadcast) last dim to an AP -> [..., count]."""
    return dataclasses.replace(ap, ap=list(ap.ap) + [[0, count]])


def _freq_tables():
    d = HD // 4
    base = (np.linspace(1.0, (HF * WF) / 2.0, d // 2, dtype=np.float64) * np.pi)
    posH = np.linspace(-1.0, 1.0, HF)
    posW = np.linspace(-1.0, 1.0, WF)
    fH = np.repeat(posH[:, None] * base[None, :], 2, axis=-1)   # [H, 16]
    fW = np.repeat(posW[:, None] * base[None, :], 2, axis=-1)   # [W, 16]
    fH = np.broadcast_to(fH[:, None, :], (HF, WF, d))
    fW = np.broadcast_to(fW[None, :, :], (HF, WF, d))
    freqs = np.concatenate([fH, fW], axis=-1).reshape(N, HD // 2)
    # freqs[:, 2i] == freqs[:, 2i+1]; keep one per pair -> [N, 16]
    half = freqs[:, 0::2].astype(np.float64)
    # [128, NT, 16]: row p, tile t -> token t*128+p
    cos = np.cos(half).astype(np.float32).reshape(NT, 128, 16).transpose(1, 0, 2)
    sin = np.sin(half).astype(np.float32).reshape(NT, 128, 16).transpose(1, 0, 2)
    return (np.ascontiguousarray(cos.reshape(128, NT * 16)),
            np.ascontiguousarray(sin.reshape(128, NT * 16)))


def _build():
    nc = bacc.Bacc("TRN2", target_bir_lowering=False, debug=False)

    xT_d = nc.dram_tensor("xT", [BPC, DIM, N], BF16, kind="ExternalInput")
    wqkvT_d = nc.dram_tensor("wqkvT", [DIM, 3 * DIM], BF16, kind="ExternalInput")
    wprojT_d = nc.dram_tensor("wprojT", [DIM, DIM], BF16, kind="ExternalInput")
    bproj_d = nc.dram_tensor("bproj", [1, DIM], BF16, kind="ExternalInput")
    ones_d = nc.dram_tensor("ones", [1, 128], BF16, kind="ExternalInput")
    cosn_d = nc.dram_tensor("cosn", [128, NT * 16], BF16, kind="ExternalInput")
    sinn_d = nc.dram_tensor("sinn", [128, NT * 16], BF16, kind="ExternalInput")
    y_d = nc.dram_tensor("y", [BPC, N, DIM], BF16, kind="ExternalOutput")

    with tile.TileContext(nc) as tc:
        with (
            tc.tile_pool(name="sb", bufs=1) as sb,
            tc.tile_pool(name="ps", bufs=1, space="PSUM") as ps,
        ):
            # ---------------- constants ----------------
            ones_r = sb.tile([1, 128], BF16, name="ones_r")
            nc.sync.dma_start(ones_r[:], ones_d.ap())
            bproj = sb.tile([1, DIM], BF16, name="bproj")
            nc.sync.dma_start(bproj[:], bproj_d.ap())
            cosn = sb.tile([128, NT * 16], BF16, name="cosn")
            sinn = sb.tile([128, NT * 16], BF16, name="sinn")
            nc.sync.dma_start(cosn[:], cosn_d.ap())
            nc.sync.dma_start(sinn[:], sinn_d.ap())
            wpr = sb.tile([128, ND, DIM], BF16, name="wpr")
            nc.sync.dma_start(
                wpr[:], wprojT_d.ap().rearrange("(c p) o -> p c o", p=128))
            # warm the exp table set early (hides the ~2.7us table load)
            expwarm = sb.tile([1, 16], F32, name="expwarm")
            nc.scalar.activation(expwarm[:], cosn[0:1, 0:16], EXP, scale=1.0)

            # ---------------- per-sample inputs ----------------
            def xT_tiles(s):
                xt = sb.tile([128, ND, N], BF16, name=f"xT_s{s}", tag="xT")
                nc.sync.dma_start(
                    xt[:],
                    xT_d.ap()[s].rearrange("(c p) n -> p c n", p=128))
                return xt

            wq_cache = {}

            def fetch_w(s, sect, half):
                """stream the [DIM, 512] weight slab for (section, half) as one
                strided DMA: [128 part, 8 in-chunk, 512 outcols]."""
                wt = sb.tile([128, ND, 512], BF16,
                             name=f"wq_s{s}_{sect}_{half}", tag="wq", bufs=2)
                nc.sync.dma_start(
                    wt[:],
                    wqkvT_d.ap().rearrange("(c p) o -> p c o", p=128)
                    [:, :, sect * DIM + half * 512: sect * DIM + half * 512 + 512])
                wq_cache[(s, sect, half)] = wt

            def proj_psum(s, xT, sect, half, t, tag):
                """[128 tok, 512 outdims] psum tile (heads half*8..half*8+8)."""
                py = ps.tile([128, 512], F32, name=f"py_{tag}", tag="mm512", bufs=2)
                wt = wq_cache[(s, sect, half)]
                for d in range(ND):
                    nc.tensor.matmul(
                        py[:],
                        xT[:, d, t * 128:(t + 1) * 128],
                        wt[:, d, :],
                        start=(d == 0), stop=(d == ND - 1))
                return py

            def v_half(s, half, t, xT, vt):
                py = proj_psum(s, xT, 2, half, t, f"v{s}{half}{t}")
                vv = vt[:].rearrange("p (h c) -> p h c", c=HD + 1)
                h0 = half * 8
                nc.vector.memset(vv[:, h0:h0 + 8, HD], 1.0)
                nc.vector.tensor_copy(
                    vv[:, h0:h0 + 8, 0:HD],
                    py[:].rearrange("p (h c) -> p h c", c=HD))
                return vt

            def qk_half(s, sect, half, t, xT, dstT):
                """project half of q (sect=0) or k (sect=1) for token-tile t,
                rotary, transpose into dstT[:, t, half*4:(half+1)*4, :]."""
                py = proj_psum(s, xT, sect, half, t, f"s{sect}_{s}{half}{t}")
                pr = py[:].rearrange("p (h i u) -> p h i u", h=8, i=32, u=2)
                ev, od = pr[:, :, 0:16, 0], pr[:, :, 0:16, 1]
                cb = _bcast_mid(cosn[:, t * 16:(t + 1) * 16], 8)
                sbb = _bcast_mid(sinn[:, t * 16:(t + 1) * 16], 8)
                qn = sb.tile([128, 512], BF16, name=f"qn_{sect}_{s}{half}{t}",
                             tag="qn", bufs=2)
                qr = qn[:].rearrange("p (h i u) -> p h i u", h=8, i=32, u=2)
                t0 = sb.tile([128, 8, 16], BF16, name=f"t0_{sect}_{s}{half}{t}",
                             tag="rt0", bufs=2)
                t1 = sb.tile([128, 8, 16], BF16, name=f"t1_{sect}_{s}{half}{t}",
                             tag="rt1", bufs=2)
                nc.vector.tensor_tensor(t0[:], ev, cb, mul)
                nc.vector.tensor_tensor(t1[:], od, sbb, mul)
                nc.vector.tensor_tensor(qr[:, :, 0:16, 0], t0[:], t1[:], sub)
                t2 = sb.tile([128, 8, 16], BF16, name=f"t2_{sect}_{s}{half}{t}",
                             tag="rt0", bufs=2)
                t3 = sb.tile([128, 8, 16], BF16, name=f"t3_{sect}_{s}{half}{t}",
                             tag="rt1", bufs=2)
                nc.vector.tensor_tensor(t2[:], od, cb, mul)
                nc.vector.tensor_tensor(t3[:], ev, sbb, mul)
                nc.vector.tensor_tensor(qr[:, :, 0:16, 1], t2[:], t3[:], add)
                # pass-through dims 32:64 of each head
                pp = py[:].rearrange("p (h c) -> p h c", c=HD)
                qp = qn[:].rearrange("p (h c) -> p h c", c=HD)
                nc.vector.tensor_copy(qp[:, :, 32:64], pp[:, :, 32:64])
                # transpose [tok, dim-half] -> qT[:, t, half*4:(half+1)*4, :]
                nc.sync.dma_start_transpose(
                    dstT[:].rearrange("p (t c q) -> p t c q", t=NT, c=ND)
                    [:, t, half * 4:(half + 1) * 4, :],
                    qn[:])

            # ---------------- attention ----------------
            def attention_head(s, h, qTt, kTt, vts, at):
                kv = kTt[:].rearrange("p (t c q) -> p t c q", t=NT, c=ND)
                qv = qTt[:].rearrange("p (t c q) -> p t c q", t=NT, c=ND)
                r0 = (h % 2) * 64
                for nch in range(2):
                    p_q = [sb.tile([128, 4, 512], BF16,
                                   name=f"p_s{s}h{h}n{nch}q{i}", tag="p", bufs=3)
                           for i in range(2)]
                    for mp in range(NT // 2):
                        st = ps.tile([128, 1024], F32, tag="st", bufs=2,
                                     name=f"st_{s}_{h}_{nch}_{mp}")
                        for u in range(2):
                            nc.tensor.matmul(
                                st[:, u * 512:(u + 1) * 512],
                                kv[r0:r0 + 64, mp * 2 + u, h // 2],
                                qv[r0:r0 + 64, nch * 4:(nch + 1) * 4, h // 2],
                            )
                        nc.scalar.activation(
                            p_q[mp // 2][:, (mp % 2) * 2:(mp % 2) * 2 + 2, :],
                            st[:], EXP, scale=float(SCALE))
                    pv = ps.tile([128, 260], F32, name=f"pv_{s}_{h}_{nch}",
                                 tag="pv", bufs=2)
                    for ql in range(4):
                        for m in range(NT):
                            nc.tensor.matmul(
                                pv[:, ql * 65:ql * 65 + 65],
                                p_q[m // 4][:, m % 4, ql * 128:(ql + 1) * 128],
                                vts[m][:].rearrange("p (h c) -> p h c",
                                                    c=HD + 1)[:, h],
                                start=(m == 0), stop=(m == NT - 1))
                    pvv = pv[:].rearrange("p (q c) -> p q c", c=65)
                    rc = sb.tile([128, 4], F32, name=f"rc_{s}_{h}_{nch}",
                                 tag="rc", bufs=2)
                    nc.vector.reciprocal_approx_fast(rc[:], pvv[:, :, 64])
                    av = at[:].rearrange("p (q h c) -> p q h c", q=NT, h=NH)
                    nc.vector.tensor_tensor(
                        av[:, nch * 4:(nch + 1) * 4, h, :],
                        pvv[:, :, 0:64], _bcast_last(rc[:], HD), mul)

            # ---------------- output projection ----------------
            def proj_out(s, qt, at):
                atq = sb.tile([128, ND, 128], BF16, name=f"atT_{s}_{qt}",
                              tag="attnT", bufs=3)
                nc.sync.dma_start_transpose(
                    atq[:], at[:, qt * 1024:(qt + 1) * 1024])
                for half in range(2):
                    py = ps.tile([128, 512], F32, name=f"yp_{s}_{qt}_{half}",
                                 tag="mm512", bufs=2)
                    for d in range(ND):
                        nc.tensor.matmul(
                            py[:],
                            atq[:, d, :],
                            wpr[:, d, half * 512:(half + 1) * 512],
                            start=(d == 0), stop=False)
                    nc.tensor.matmul(
                        py[:],
                        ones_r[:], bproj[:, half * 512:(half + 1) * 512],
                        start=False, stop=True)
                    ysb = sb.tile([128, 512], BF16, name=f"y_{s}_{qt}_{half}",
                                  tag="ysb", bufs=1)
                    nc.vector.tensor_copy(ysb[:], py[:])
                    nc.sync.dma_start(
                        y_d.ap()[s, qt * 128:(qt + 1) * 128,
                                 half * 512:(half + 1) * 512],
                        ysb[:])

            # ================= emission schedule =================
            qT = [sb.tile([128, NT * ND * 128], BF16, name=f"qT_s{s}", tag="qT",
                          bufs=2) for s in range(BPC)]
            kT = [sb.tile([128, NT * ND * 128], BF16, name=f"kT_s{s}", tag="kT",
                          bufs=2) for s in range(BPC)]
            attn = [sb.tile([128, NT * 1024], BF16, name=f"attn_s{s}", tag="attn",
                            bufs=2) for s in range(BPC)]
            vsb = [[sb.tile([128, NH * (HD + 1)], BF16, name=f"v_s{s}_{t}",
                            tag=f"v{t}", bufs=2) for t in range(NT)]
                   for s in range(BPC)]

            def slab_items(s, sect, half, xT):
                """fetch + the 8 per-t work items for one weight slab."""
                items = [("w", (s, sect, half))]
                for t in range(NT):
                    if sect == 2:
                        items.append(("v", (s, half, t, xT)))
                    else:
                        items.append(("qk", (s, sect, half, t, xT)))
                return items

            def run_item(it):
                kind, args = it
                if kind == "w":
                    fetch_w(*args)
                elif kind == "v":
                    s_, half, t, xT = args
                    v_half(s_, half, t, xT, vsb[s_][t])
                else:
                    s_, sect, half, t, xT = args
                    qk_half(s_, sect, half, t, xT,
                            qT[s_] if sect == 0 else kT[s_])

            def run_slabs(slabs):
                """emit slab work with fetches hoisted 2 slabs ahead."""
                items = []
                for i, (s_, sect, xT) in enumerate(slabs):
                    for half in range(2):
                        items.append(slab_items(s_, sect, half, xT))
                # reorder: fetch of slab i+2 goes before slab i's t-work
                out = []
                fetched = 0
                for i in range(len(items)):
                    while fetched <= min(i + 2, len(items) - 1):
                        out.append(items[fetched][0]); fetched += 1
                    out.extend(items[i][1:])
                return out

            # ---- phase P(s0): full projection of sample 0 (V, K, Q) ----
            xT0 = xT_tiles(0)
            for it in run_slabs([(0, 2, xT0), (0, 1, xT0), (0, 0, xT0)]):
                run_item(it)

            # ---- phase A(s0) interleaved with full projection of s1 ----
            xT1 = sb.tile([128, ND, N], BF16, name="xT_s1", tag="xT")
            s1_filler = run_slabs([(1, 2, xT1), (1, 1, xT1), (1, 0, xT1)])
            # first two s1 weight slabs go on the queue before the big xT1
            # DMA (which must wait for P(s0) to release the xT slot)
            run_item(s1_filler[0])
            run_item(s1_filler[1])
            nc.sync.dma_start(
                xT1[:], xT_d.ap()[1].rearrange("(c p) n -> p c n", p=128))
            fi = 2
            for h in range(NH):
                attention_head(0, h, qT[0], kT[0], vsb[0], attn[0])
                take = (len(s1_filler) * (h + 1)) // NH - fi
                for _ in range(take):
                    run_item(s1_filler[fi]); fi += 1

            # ---- phase A(s1) interleaved with out-proj of s0 ----
            for h in range(NH):
                attention_head(1, h, qT[1], kT[1], vsb[1], attn[1])
                if h % 2 == 1:
                    proj_out(0, h // 2, attn[0])

            # ---- out-proj of s1 ----
            for qt in range(NT):
                proj_out(1, qt, attn[1])

    nc.compile()
    return nc


_NC_CACHE = None


def kernel(x, w_qkv, w_proj, b_proj):
    global _NC_CACHE, last_exec_time_ns
    import ml_dtypes

    x = np.asarray(x, np.float32)
    w_qkv = np.asarray(w_qkv, np.float32)
    w_proj = np.asarray(w_proj, np.float32)
    b_proj = np.asarray(b_proj, np.float32)

    if _NC_CACHE is None:
        _NC_CACHE = _build()
    nc = _NC_CACHE

    cosn, sinn = _freq_tables()
    cosn = cosn.astype(ml_dtypes.bfloat16)
    sinn = sinn.astype(ml_dtypes.bfloat16)
    wqkvT = np.ascontiguousarray(w_qkv.T).astype(ml_dtypes.bfloat16)
    wprojT = np.ascontiguousarray(w_proj.T).astype(ml_dtypes.bfloat16)
    bproj16 = b_proj.reshape(1, DIM).astype(ml_dtypes.bfloat16)
    ones16 = np.ones((1, 128), ml_dtypes.bfloat16)

    in_maps = []
    for c in range(NCORES):
        xs = x[c * BPC:(c + 1) * BPC]                       # [2, N, DIM]
        xT = np.ascontiguousarray(xs.transpose(0, 2, 1)).astype(ml_dtypes.bfloat16)
        in_maps.append({
            "xT": xT, "wqkvT": wqkvT, "wprojT": wprojT,
            "bproj": bproj16, "ones": ones16, "cosn": cosn, "sinn": sinn,
        })

    trace = bool(os.environ.get("KERNEL_TRACE"))
    kwargs = {}
    if trace:
        kwargs["trace"] = True
        td = os.environ.get("KERNEL_TRACE_DIR")
        if td:
            kwargs["tmpdir"] = td
    res = bass_utils.run_bass_kernel_spmd(
        nc, in_maps, core_ids=list(range(NCORES)), **kwargs)
    last_exec_time_ns = res.exec_time_ns
    out = np.concatenate([np.asarray(res.results[c]["y"]) for c in range(NCORES)],
                         axis=0)
    return np.ascontiguousarray(out.reshape(B, N, DIM).astype(np.float32))


if __name__ == "__main__":
    rng = np.random.default_rng(0)
    xs = rng.standard_normal((B, N, DIM), dtype=np.float32)
    wq = rng.standard_normal((3 * DIM, DIM), dtype=np.float32) / 32
    wp = rng.standard_normal((DIM, DIM), dtype=np.float32) / 32
    bp = np.zeros(DIM, np.float32)
    y = kernel(xs, wq, wp, bp)
    print("y", y.shape, y.dtype, float(np.abs(y).max()))


# revision 9
# speedup vs baseline: 1.8570x; 1.0420x over previous
"""Trainium2 Bass kernel for nn_Attention_13348758356565.

Dense transformer attention block (B=16, N=1024 tokens, DIM=1024, 16 heads x 64)
with axial rotary embeddings, data-parallel over batch across 8 NeuronCores
(2 samples per core). All matmuls bf16 on TensorE at full rate.

Per sample:
- QKV projection x-stationary -> natural [tok, outdim] psum tiles [128, 512]
  (8 heads per half). Rotary applied by DVE directly from psum; V drains into
  [keys, 16*(64+1)] tiles with an interleaved ones column per head (free
  softmax denominator).
- Q/K/attn transposes are dma_start_transpose (xbar) calls: no PE transposes,
  no psum-drain copies. qT/kT layout: [dim%128, (t, dim//128, tok%128)].
- QK^T: kT-slice stationary [64, 128], qT moving [64, 4, 128] -> scores
  [keys, queries] psum; exp on ScalarE -> p bf16 in SBUF.
- P*V: p-chunk stationary [128 keys, 128 queries], moving V[keys, 65] ->
  [queries, 65] psum accumulated over key tiles at full PE rate; denominator
  is column 64 -> normalized by one broadcast tensor_tensor into natural
  attn layout (no partition broadcast needed).
- out-proj: attnT-stationary chunks vs wprojT moving; y written bf16 and
  upcast on host.

Cross-sample emission interleave keeps the PE stream dense during the
exp-gated attention phase so the HAM clock gate stays at 2.4 GHz.
"""

import os
import sys

sys.path.insert(0, "/opt/trn_rl_repo")

import dataclasses
import numpy as np

import concourse.bacc as bacc
import concourse.mybir as mybir
import concourse.tile as tile
from concourse import bass_utils

F32 = mybir.dt.float32
BF16 = mybir.dt.bfloat16
EXP = mybir.ActivationFunctionType.Exp

B, HF, WF = 16, 32, 32
DIM, NH, HD = 1024, 16, 64
N = HF * WF          # 1024 tokens
NCORES = 8
BPC = B // NCORES    # 2 samples per core
NT = N // 128        # 8 token tiles
ND = DIM // 128      # 8 contraction chunks
SCALE = 1.0 / np.sqrt(HD)

mul = mybir.AluOpType.mult
sub = mybir.AluOpType.subtract
add = mybir.AluOpType.add

last_exec_time_ns = None


def _bcast_mid(ap, count):
    """Insert a step-0 (broadcast) middle dim into a [P, C] AP -> [P, count, C]."""
    return dataclasses.replace(ap, ap=[ap.ap[0], [0, count], ap.ap[1]])


def _bcast_last(ap, count):
    """Append a step-0 (broadcast) last dim to an AP -> [..., count]."""
    return dataclasses.replace(ap, ap=list(ap.ap) + [[0, count]])


def _freq_tables():
    d = HD // 4
    base = (np.linspace(1.0, (HF * WF) / 2.0, d // 2, dtype=np.float64) * np.pi)
    posH = np.linspace(-1.0, 1.0, HF)
    posW = np.linspace(-1.0, 1.0, WF)
    fH = np.repeat(posH[:, None] * base[None, :], 2, axis=-1)   # [H, 16]
    fW = np.repeat(posW[:, None] * base[None, :], 2, axis=-1)   # [W, 16]
    fH = np.broadcast_to(fH[:, None, :], (HF, WF, d))
    fW = np.broadcast_to(fW[None, :, :], (HF, WF, d))
    freqs = np.concatenate([fH, fW], axis=-1).reshape(N, HD // 2)
    # freqs[:, 2i] == freqs[:, 2i+1]; keep one per pair -> [N, 16]
    half = freqs[:, 0::2].astype(np.float64)
    # [128, NT, 16]: row p, tile t -> token t*128+p
    cos = np.cos(half).astype(np.float32).reshape(NT, 128, 16).transpose(1, 0, 2)
    sin = np.sin(half).astype(np.float32).reshape(NT, 128, 16).transpose(1, 0, 2)
    return (np.ascontiguousarray(cos.reshape(128, NT * 16)),
            np.ascontiguousarray(sin.reshape(128, NT * 16)))


def _build():
    nc = bacc.Bacc("TRN2", target_bir_lowering=False, debug=False)

    xT_d = nc.dram_tensor("xT", [BPC, DIM, N], BF16, kind="ExternalInput")
    wqkvT_d = nc.dram_tensor("wqkvT", [DIM, 3 * DIM], BF16, kind="ExternalInput")
    wprojT_d = nc.dram_tensor("wprojT", [DIM, DIM], BF16, kind="ExternalInput")
    bproj_d = nc.dram_tensor("bproj", [1, DIM], BF16, kind="ExternalInput")
    ones_d = nc.dram_tensor("ones", [1, 128], BF16, kind="ExternalInput")
    cosn_d = nc.dram_tensor("cosn", [128, NT * 16], BF16, kind="ExternalInput")
    sinn_d = nc.dram_tensor("sinn", [128, NT * 16], BF16, kind="ExternalInput")
    y_d = nc.dram_tensor("y", [BPC, N, DIM], BF16, kind="ExternalOutput")

    with tile.TileContext(nc) as tc:
        with (
            tc.tile_pool(name="sb", bufs=1) as sb,
            tc.tile_pool(name="ps", bufs=1, space="PSUM") as ps,
        ):
            # ---------------- constants ----------------
            ones_r = sb.tile([1, 128], BF16, name="ones_r")
            nc.sync.dma_start(ones_r[:], ones_d.ap())
            bproj = sb.tile([1, DIM], BF16, name="bproj")
            nc.sync.dma_start(bproj[:], bproj_d.ap())
            cosn = sb.tile([128, NT * 16], BF16, name="cosn")
            sinn = sb.tile([128, NT * 16], BF16, name="sinn")
            nc.sync.dma_start(cosn[:], cosn_d.ap())
            nc.sync.dma_start(sinn[:], sinn_d.ap())
            wpr = sb.tile([128, ND, DIM], BF16, name="wpr")
            nc.sync.dma_start(
                wpr[:], wprojT_d.ap().rearrange("(c p) o -> p c o", p=128))
            # warm the exp table set early (hides the ~2.7us table load)
            expwarm = sb.tile([1, 16], F32, name="expwarm")
            nc.scalar.activation(expwarm[:], cosn[0:1, 0:16], EXP, scale=1.0)

            # ---------------- per-sample inputs ----------------
            def xT_tiles(s):
                xt = sb.tile([128, ND, N], BF16, name=f"xT_s{s}", tag="xT")
                nc.sync.dma_start(
                    xt[:],
                    xT_d.ap()[s].rearrange("(c p) n -> p c n", p=128))
                return xt

            wq_cache = {}

            def fetch_w(s, sect, half):
                """stream the [DIM, 512] weight slab for (section, half) as one
                strided DMA: [128 part, 8 in-chunk, 512 outcols]."""
                wt = sb.tile([128, ND, 512], BF16,
                             name=f"wq_s{s}_{sect}_{half}", tag="wq", bufs=2)
                nc.sync.dma_start(
                    wt[:],
                    wqkvT_d.ap().rearrange("(c p) o -> p c o", p=128)
                    [:, :, sect * DIM + half * 512: sect * DIM + half * 512 + 512])
                wq_cache[(s, sect, half)] = wt

            def proj_psum(s, xT, sect, half, t, tag):
                """[128 tok, 512 outdims] psum tile (heads half*8..half*8+8)."""
                py = ps.tile([128, 512], F32, name=f"py_{tag}", tag="mm512", bufs=2)
                wt = wq_cache[(s, sect, half)]
                for d in range(ND):
                    nc.tensor.matmul(
                        py[:],
                        xT[:, d, t * 128:(t + 1) * 128],
                        wt[:, d, :],
                        start=(d == 0), stop=(d == ND - 1))
                return py

            def v_half(s, half, t, xT, vt):
                py = proj_psum(s, xT, 2, half, t, f"v{s}{half}{t}")
                vv = vt[:].rearrange("p (h c) -> p h c", c=HD + 1)
                h0 = half * 8
                nc.vector.memset(vv[:, h0:h0 + 8, HD], 1.0)
                nc.vector.tensor_copy(
                    vv[:, h0:h0 + 8, 0:HD],
                    py[:].rearrange("p (h c) -> p h c", c=HD))
                return vt

            def qk_half(s, sect, half, t, xT, dstT):
                """project half of q (sect=0) or k (sect=1) for token-tile t,
                rotary, transpose into dstT[:, t, half*4:(half+1)*4, :]."""
                py = proj_psum(s, xT, sect, half, t, f"s{sect}_{s}{half}{t}")
                pr = py[:].rearrange("p (h i u) -> p h i u", h=8, i=32, u=2)
                ev, od = pr[:, :, 0:16, 0], pr[:, :, 0:16, 1]
                cb = _bcast_mid(cosn[:, t * 16:(t + 1) * 16], 8)
                sbb = _bcast_mid(sinn[:, t * 16:(t + 1) * 16], 8)
                qn = sb.tile([128, 512], BF16, name=f"qn_{sect}_{s}{half}{t}",
                             tag="qn", bufs=2)
                qr = qn[:].rearrange("p (h i u) -> p h i u", h=8, i=32, u=2)
                t0 = sb.tile([128, 8, 16], BF16, name=f"t0_{sect}_{s}{half}{t}",
                             tag="rt0", bufs=2)
                t1 = sb.tile([128, 8, 16], BF16, name=f"t1_{sect}_{s}{half}{t}",
                             tag="rt1", bufs=2)
                nc.vector.tensor_tensor(t0[:], ev, cb, mul)
                nc.vector.tensor_tensor(t1[:], od, sbb, mul)
                nc.vector.tensor_tensor(qr[:, :, 0:16, 0], t0[:], t1[:], sub)
                t2 = sb.tile([128, 8, 16], BF16, name=f"t2_{sect}_{s}{half}{t}",
                             tag="rt0", bufs=2)
                t3 = sb.tile([128, 8, 16], BF16, name=f"t3_{sect}_{s}{half}{t}",
                             tag="rt1", bufs=2)
                nc.vector.tensor_tensor(t2[:], od, cb, mul)
                nc.vector.tensor_tensor(t3[:], ev, sbb, mul)
                nc.vector.tensor_tensor(qr[:, :, 0:16, 1], t2[:], t3[:], add)
                # pass-through dims 32:64 of each head
                pp = py[:].rearrange("p (h c) -> p h c", c=HD)
                qp = qn[:].rearrange("p (h c) -> p h c", c=HD)
                nc.vector.tensor_copy(qp[:, :, 32:64], pp[:, :, 32:64])
                # transpose [tok, dim-half] -> qT[:, t, half*4:(half+1)*4, :]
                nc.sync.dma_start_transpose(
                    dstT[:].rearrange("p (t c q) -> p t c q", t=NT, c=ND)
                    [:, t, half * 4:(half + 1) * 4, :],
                    qn[:])

            # ---------------- attention ----------------
            def attention_head(s, h, qTt, kTt, vts, at):
                kv = kTt[:].rearrange("p (t c q) -> p t c q", t=NT, c=ND)
                qv = qTt[:].rearrange("p (t c q) -> p t c q", t=NT, c=ND)
                r0 = (h % 2) * 64
                for nch in range(2):
                    p_q = [sb.tile([128, 4, 512], BF16,
                                   name=f"p_s{s}h{h}n{nch}q{i}", tag="p", bufs=3)
                           for i in range(2)]
                    for mp in range(NT // 2):
                        st = ps.tile([128, 1024], F32, tag="st", bufs=2,
                                     name=f"st_{s}_{h}_{nch}_{mp}")
                        for u in range(2):
                            nc.tensor.matmul(
                                st[:, u * 512:(u + 1) * 512],
                                kv[r0:r0 + 64, mp * 2 + u, h // 2],
                                qv[r0:r0 + 64, nch * 4:(nch + 1) * 4, h // 2],
                            )
                        nc.scalar.activation(
                            p_q[mp // 2][:, (mp % 2) * 2:(mp % 2) * 2 + 2, :],
                            st[:], EXP, scale=float(SCALE))
                    pv = ps.tile([128, 260], F32, name=f"pv_{s}_{h}_{nch}",
                                 tag="pv", bufs=2)
                    for ql in range(4):
                        for m in range(NT):
                            nc.tensor.matmul(
                                pv[:, ql * 65:ql * 65 + 65],
                                p_q[m // 4][:, m % 4, ql * 128:(ql + 1) * 128],
                                vts[m][:].rearrange("p (h c) -> p h c",
                                                    c=HD + 1)[:, h],
                                start=(m == 0), stop=(m == NT - 1))
                    pvv = pv[:].rearrange("p (q c) -> p q c", c=65)
                    rc = sb.tile([128, 4], F32, name=f"rc_{s}_{h}_{nch}",
                                 tag="rc", bufs=2)
                    nc.vector.reciprocal_approx_fast(rc[:], pvv[:, :, 64])
                    av = at[:].rearrange("p (q h c) -> p q h c", q=NT, h=NH)
                    nc.vector.tensor_tensor(
                        av[:, nch * 4:(nch + 1) * 4, h, :],
                        pvv[:, :, 0:64], _bcast_last(rc[:], HD), mul)

            # ---------------- output projection ----------------
            def proj_out(s, qt, at):
                atq = sb.tile([128, ND, 128], BF16, name=f"atT_{s}_{qt}",
                              tag="attnT", bufs=3)
                nc.sync.dma_start_transpose(
                    atq[:], at[:, qt * 1024:(qt + 1) * 1024])
                for half in range(2):
                    py = ps.tile([128, 512], F32, name=f"yp_{s}_{qt}_{half}",
                                 tag="mm512", bufs=2)
                    for d in range(ND):
                        nc.tensor.matmul(
                            py[:],
                            atq[:, d, :],
                            wpr[:, d, half * 512:(half + 1) * 512],
                            start=(d == 0), stop=False)
                    nc.tensor.matmul(
                        py[:],
                        ones_r[:], bproj[:, half * 512:(half + 1) * 512],
                        start=False, stop=True)
                    ysb = sb.tile([128, 512], BF16, name=f"y_{s}_{qt}_{half}",
                                  tag="ysb", bufs=1)
                    nc.vector.tensor_copy(ysb[:], py[:])
                    nc.sync.dma_start(
                        y_d.ap()[s, qt * 128:(qt + 1) * 128,
                                 half * 512:(half + 1) * 512],
                        ysb[:])

            # ================= emission schedule =================
            qT = [sb.tile([128, NT * ND * 128], BF16, name=f"qT_s{s}", tag="qT",
                          bufs=2) for s in range(BPC)]
            kT = [sb.tile([128, NT * ND * 128], BF16, name=f"kT_s{s}", tag="kT",
                          bufs=2) for s in range(BPC)]
            attn = [sb.tile([128, NT * 1024], BF16, name=f"attn_s{s}", tag="attn",
                            bufs=2) for s in range(BPC)]
            vsb = [[sb.tile([128, NH * (HD + 1)], BF16, name=f"v_s{s}_{t}",
                            tag=f"v{t}", bufs=2) for t in range(NT)]
                   for s in range(BPC)]

            def slab_items(s, sect, half, xT):
                """fetch + the 8 per-t work items for one weight slab."""
                items = [("w", (s, sect, half))]
                for t in range(NT):
                    if sect == 2:
                        items.append(("v", (s, half, t, xT)))
                    else:
                        items.append(("qk", (s, sect, half, t, xT)))
                return items

            def run_item(it):
                kind, args = it
                if kind == "w":
                    fetch_w(*args)
                elif kind == "v":
                    s_, half, t, xT = args
                    v_half(s_, half, t, xT, vsb[s_][t])
                else:
                    s_, sect, half, t, xT = args
                    qk_half(s_, sect, half, t, xT,
                            qT[s_] if sect == 0 else kT[s_])

            def run_slabs(slabs):
                """emit slab work with fetches hoisted 2 slabs ahead."""
                items = [slab_items(s_, sect, half, xT)
                         for (s_, sect, half, xT) in slabs]
                # reorder: fetch of slab i+2 goes before slab i's t-work
                out = []
                fetched = 0
                for i in range(len(items)):
                    while fetched <= min(i + 2, len(items) - 1):
                        out.append(items[fetched][0]); fetched += 1
                    out.extend(items[i][1:])
                return out

            # ---- phase P(s0): full projection of sample 0 (V, K, Q) ----
            xT0 = xT_tiles(0)
            for it in run_slabs([(0, 2, 0, xT0), (0, 2, 1, xT0),
                                 (0, 1, 0, xT0), (0, 1, 1, xT0),
                                 (0, 0, 0, xT0), (0, 0, 1, xT0)]):
                run_item(it)

            # ---- phase A(s0): s0 attention + s1 V / K-half0 / Q-half0 ----
            xT1 = sb.tile([128, ND, N], BF16, name="xT_s1", tag="xT")
            s1_filler = run_slabs([(1, 2, 0, xT1), (1, 2, 1, xT1),
                                   (1, 1, 0, xT1), (1, 0, 0, xT1)])
            # first two s1 weight slabs go on the queue before the big xT1
            # DMA (which must wait for P(s0) to release the xT slot)
            run_item(s1_filler[0])
            run_item(s1_filler[1])
            nc.sync.dma_start(
                xT1[:], xT_d.ap()[1].rearrange("(c p) n -> p c n", p=128))
            fi = 2
            for h in range(NH):
                attention_head(0, h, qT[0], kT[0], vsb[0], attn[0])
                take = (len(s1_filler) * (h + 1)) // NH - fi
                for _ in range(take):
                    run_item(s1_filler[fi]); fi += 1

            # ---- phase A(s1): s1 attention (heads 0-7 ready) + s1 K/Q
            # half1 projection + out-proj of s0 ----
            late = run_slabs([(1, 1, 1, xT1), (1, 0, 1, xT1)])
            li = 0
            for h in range(NH):
                attention_head(1, h, qT[1], kT[1], vsb[1], attn[1])
                if h < 6:
                    take = (len(late) * (h + 1)) // 6 - li
                    for _ in range(take):
                        run_item(late[li]); li += 1
                if h % 2 == 1:
                    proj_out(0, h // 2, attn[0])

            # ---- out-proj of s1 ----
            for qt in range(NT):
                proj_out(1, qt, attn[1])

    nc.compile()
    return nc


_NC_CACHE = None


def kernel(x, w_qkv, w_proj, b_proj):
    global _NC_CACHE, last_exec_time_ns
    import ml_dtypes

    x = np.asarray(x, np.float32)
    w_qkv = np.asarray(w_qkv, np.float32)
    w_proj = np.asarray(w_proj, np.float32)
    b_proj = np.asarray(b_proj, np.float32)

    if _NC_CACHE is None:
        _NC_CACHE = _build()
    nc = _NC_CACHE

    cosn, sinn = _freq_tables()
    cosn = cosn.astype(ml_dtypes.bfloat16)
    sinn = sinn.astype(ml_dtypes.bfloat16)
    wqkvT = np.ascontiguousarray(w_qkv.T).astype(ml_dtypes.bfloat16)
    wprojT = np.ascontiguousarray(w_proj.T).astype(ml_dtypes.bfloat16)
    bproj16 = b_proj.reshape(1, DIM).astype(ml_dtypes.bfloat16)
    ones16 = np.ones((1, 128), ml_dtypes.bfloat16)

    in_maps = []
    for c in range(NCORES):
        xs = x[c * BPC:(c + 1) * BPC]                       # [2, N, DIM]
        xT = np.ascontiguousarray(xs.transpose(0, 2, 1)).astype(ml_dtypes.bfloat16)
        in_maps.append({
            "xT": xT, "wqkvT": wqkvT, "wprojT": wprojT,
            "bproj": bproj16, "ones": ones16, "cosn": cosn, "sinn": sinn,
        })

    trace = bool(os.environ.get("KERNEL_TRACE"))
    kwargs = {}
    if trace:
        kwargs["trace"] = True
        td = os.environ.get("KERNEL_TRACE_DIR")
        if td:
            kwargs["tmpdir"] = td
    res = bass_utils.run_bass_kernel_spmd(
        nc, in_maps, core_ids=list(range(NCORES)), **kwargs)
    last_exec_time_ns = res.exec_time_ns
    out = np.concatenate([np.asarray(res.results[c]["y"]) for c in range(NCORES)],
                         axis=0)
    return np.ascontiguousarray(out.reshape(B, N, DIM).astype(np.float32))


if __name__ == "__main__":
    rng = np.random.default_rng(0)
    xs = rng.standard_normal((B, N, DIM), dtype=np.float32)
    wq = rng.standard_normal((3 * DIM, DIM), dtype=np.float32) / 32
    wp = rng.standard_normal((DIM, DIM), dtype=np.float32) / 32
    bp = np.zeros(DIM, np.float32)
    y = kernel(xs, wq, wp, bp)
    print("y", y.shape, y.dtype, float(np.abs(y).max()))


# revision 10
# speedup vs baseline: 1.9563x; 1.0535x over previous
"""Trainium2 Bass kernel for nn_Attention_13348758356565.

Dense transformer attention block (B=16, N=1024 tokens, DIM=1024, 16 heads x 64)
with axial rotary embeddings, data-parallel over batch across 8 NeuronCores
(2 samples per core). All matmuls bf16 on TensorE at full rate.

Per sample:
- QKV projection x-stationary -> natural [tok, outdim] psum tiles [128, 512]
  (8 heads per half). Rotary applied by DVE directly from psum; V drains into
  [keys, 16*(64+1)] tiles with an interleaved ones column per head (free
  softmax denominator).
- Q/K/attn transposes are dma_start_transpose (xbar) calls: no PE transposes,
  no psum-drain copies. qT/kT layout: [dim%128, (t, dim//128, tok%128)].
- QK^T: kT-slice stationary [64, 128], qT moving [64, 4, 128] -> scores
  [keys, queries] psum; exp on ScalarE -> p bf16 in SBUF.
- P*V: p-chunk stationary [128 keys, 128 queries], moving V[keys, 65] ->
  [queries, 65] psum accumulated over key tiles at full PE rate; denominator
  is column 64 -> normalized by one broadcast tensor_tensor into natural
  attn layout (no partition broadcast needed).
- out-proj: attnT-stationary chunks vs wprojT moving; y written bf16 and
  upcast on host.

Cross-sample emission interleave keeps the PE stream dense during the
exp-gated attention phase so the HAM clock gate stays at 2.4 GHz.
"""

import os
import sys

sys.path.insert(0, "/opt/trn_rl_repo")

import dataclasses
import numpy as np

import concourse.bacc as bacc
import concourse.mybir as mybir
import concourse.tile as tile
from concourse import bass_utils

F32 = mybir.dt.float32
BF16 = mybir.dt.bfloat16
EXP = mybir.ActivationFunctionType.Exp

B, HF, WF = 16, 32, 32
DIM, NH, HD = 1024, 16, 64
N = HF * WF          # 1024 tokens
NCORES = 8
BPC = B // NCORES    # 2 samples per core
NT = N // 128        # 8 token tiles
ND = DIM // 128      # 8 contraction chunks
SCALE = 1.0 / np.sqrt(HD)

mul = mybir.AluOpType.mult
sub = mybir.AluOpType.subtract
add = mybir.AluOpType.add

last_exec_time_ns = None


def _bcast_mid(ap, count):
    """Insert a step-0 (broadcast) middle dim into a [P, C] AP -> [P, count, C]."""
    return dataclasses.replace(ap, ap=[ap.ap[0], [0, count], ap.ap[1]])


def _bcast_last(ap, count):
    """Append a step-0 (broadcast) last dim to an AP -> [..., count]."""
    return dataclasses.replace(ap, ap=list(ap.ap) + [[0, count]])


def _freq_tables():
    d = HD // 4
    base = (np.linspace(1.0, (HF * WF) / 2.0, d // 2, dtype=np.float64) * np.pi)
    posH = np.linspace(-1.0, 1.0, HF)
    posW = np.linspace(-1.0, 1.0, WF)
    fH = np.repeat(posH[:, None] * base[None, :], 2, axis=-1)   # [H, 16]
    fW = np.repeat(posW[:, None] * base[None, :], 2, axis=-1)   # [W, 16]
    fH = np.broadcast_to(fH[:, None, :], (HF, WF, d))
    fW = np.broadcast_to(fW[None, :, :], (HF, WF, d))
    freqs = np.concatenate([fH, fW], axis=-1).reshape(N, HD // 2)
    # freqs[:, 2i] == freqs[:, 2i+1]; keep one per pair -> [N, 16]
    half = freqs[:, 0::2].astype(np.float64)
    # [128, NT, 16]: row p, tile t -> token t*128+p
    cos = np.cos(half).astype(np.float32).reshape(NT, 128, 16).transpose(1, 0, 2)
    sin = np.sin(half).astype(np.float32).reshape(NT, 128, 16).transpose(1, 0, 2)
    return (np.ascontiguousarray(cos.reshape(128, NT * 16)),
            np.ascontiguousarray(sin.reshape(128, NT * 16)))


def _build():
    nc = bacc.Bacc("TRN2", target_bir_lowering=False, debug=False)

    xT_d = nc.dram_tensor("xT", [BPC, DIM, N], BF16, kind="ExternalInput")
    wqkvT_d = nc.dram_tensor("wqkvT", [DIM, 3 * DIM], BF16, kind="ExternalInput")
    wprojT_d = nc.dram_tensor("wprojT", [DIM, DIM], BF16, kind="ExternalInput")
    bproj_d = nc.dram_tensor("bproj", [1, DIM], BF16, kind="ExternalInput")
    ones_d = nc.dram_tensor("ones", [1, 128], BF16, kind="ExternalInput")
    cosn_d = nc.dram_tensor("cosn", [128, NT * 16], BF16, kind="ExternalInput")
    sinn_d = nc.dram_tensor("sinn", [128, NT * 16], BF16, kind="ExternalInput")
    y_d = nc.dram_tensor("y", [BPC, N, DIM], BF16, kind="ExternalOutput")

    with tile.TileContext(nc) as tc:
        with (
            tc.tile_pool(name="sb", bufs=1) as sb,
            tc.tile_pool(name="ps", bufs=1, space="PSUM") as ps,
        ):
            # ---------------- constants ----------------
            ones_r = sb.tile([1, 128], BF16, name="ones_r")
            nc.scalar.dma_start(ones_r[:], ones_d.ap())
            bproj = sb.tile([1, DIM], BF16, name="bproj")
            nc.scalar.dma_start(bproj[:], bproj_d.ap())
            cosn = sb.tile([128, NT * 16], BF16, name="cosn")
            sinn = sb.tile([128, NT * 16], BF16, name="sinn")
            nc.scalar.dma_start(cosn[:], cosn_d.ap())
            nc.scalar.dma_start(sinn[:], sinn_d.ap())
            wpr = sb.tile([128, ND, DIM], BF16, name="wpr")
            nc.scalar.dma_start(
                wpr[:], wprojT_d.ap().rearrange("(c p) o -> p c o", p=128))
            # warm the exp table set early (hides the ~2.7us table load)
            expwarm = sb.tile([1, 16], F32, name="expwarm")
            nc.scalar.activation(expwarm[:], cosn[0:1, 0:16], EXP, scale=1.0)

            # ---------------- per-sample inputs ----------------
            def xT_tiles(s):
                xt = sb.tile([128, ND, N], BF16, name=f"xT_s{s}", tag="xT")
                nc.scalar.dma_start(
                    xt[:],
                    xT_d.ap()[s].rearrange("(c p) n -> p c n", p=128))
                return xt

            wq_cache = {}

            def fetch_w(s, sect, half, eng=None):
                """stream the [DIM, 512] weight slab for (section, half) as one
                strided DMA: [128 part, 8 in-chunk, 512 outcols]."""
                wt = sb.tile([128, ND, 512], BF16,
                             name=f"wq_s{s}_{sect}_{half}", tag="wq", bufs=2)
                (eng or nc.sync).dma_start(
                    wt[:],
                    wqkvT_d.ap().rearrange("(c p) o -> p c o", p=128)
                    [:, :, sect * DIM + half * 512: sect * DIM + half * 512 + 512])
                wq_cache[(s, sect, half)] = wt

            def proj_psum(s, xT, sect, half, t, tag):
                """[128 tok, 512 outdims] psum tile (heads half*8..half*8+8)."""
                py = ps.tile([128, 512], F32, name=f"py_{tag}", tag="mm512", bufs=2)
                wt = wq_cache[(s, sect, half)]
                for d in range(ND):
                    nc.tensor.matmul(
                        py[:],
                        xT[:, d, t * 128:(t + 1) * 128],
                        wt[:, d, :],
                        start=(d == 0), stop=(d == ND - 1))
                return py

            def v_half(s, half, t, xT, vt):
                py = proj_psum(s, xT, 2, half, t, f"v{s}{half}{t}")
                vv = vt[:].rearrange("p (h c) -> p h c", c=HD + 1)
                h0 = half * 8
                nc.vector.memset(vv[:, h0:h0 + 8, HD], 1.0)
                nc.vector.tensor_copy(
                    vv[:, h0:h0 + 8, 0:HD],
                    py[:].rearrange("p (h c) -> p h c", c=HD))
                return vt

            def qk_half(s, sect, half, t, xT, dstT):
                """project half of q (sect=0) or k (sect=1) for token-tile t,
                rotary, transpose into dstT[:, t, half*4:(half+1)*4, :]."""
                py = proj_psum(s, xT, sect, half, t, f"s{sect}_{s}{half}{t}")
                pr = py[:].rearrange("p (h i u) -> p h i u", h=8, i=32, u=2)
                ev, od = pr[:, :, 0:16, 0], pr[:, :, 0:16, 1]
                cb = _bcast_mid(cosn[:, t * 16:(t + 1) * 16], 8)
                sbb = _bcast_mid(sinn[:, t * 16:(t + 1) * 16], 8)
                qn = sb.tile([128, 512], BF16, name=f"qn_{sect}_{s}{half}{t}",
                             tag="qn", bufs=2)
                qr = qn[:].rearrange("p (h i u) -> p h i u", h=8, i=32, u=2)
                t0 = sb.tile([128, 8, 16], BF16, name=f"t0_{sect}_{s}{half}{t}",
                             tag="rt0", bufs=1)
                t1 = sb.tile([128, 8, 16], BF16, name=f"t1_{sect}_{s}{half}{t}",
                             tag="rt1", bufs=1)
                nc.vector.tensor_tensor(t0[:], ev, cb, mul)
                nc.vector.tensor_tensor(t1[:], od, sbb, mul)
                nc.vector.tensor_tensor(qr[:, :, 0:16, 0], t0[:], t1[:], sub)
                t2 = sb.tile([128, 8, 16], BF16, name=f"t2_{sect}_{s}{half}{t}",
                             tag="rt0", bufs=1)
                t3 = sb.tile([128, 8, 16], BF16, name=f"t3_{sect}_{s}{half}{t}",
                             tag="rt1", bufs=1)
                nc.vector.tensor_tensor(t2[:], od, cb, mul)
                nc.vector.tensor_tensor(t3[:], ev, sbb, mul)
                nc.vector.tensor_tensor(qr[:, :, 0:16, 1], t2[:], t3[:], add)
                # pass-through dims 32:64 of each head
                pp = py[:].rearrange("p (h c) -> p h c", c=HD)
                qp = qn[:].rearrange("p (h c) -> p h c", c=HD)
                nc.vector.tensor_copy(qp[:, :, 32:64], pp[:, :, 32:64])
                # transpose [tok, dim-half] -> qT[:, t, half*4:(half+1)*4, :]
                nc.sync.dma_start_transpose(
                    dstT[:].rearrange("p (t c q) -> p t c q", t=NT, c=ND)
                    [:, t, half * 4:(half + 1) * 4, :],
                    qn[:])

            # ---------------- attention ----------------
            def attention_head(s, h, qTt, kTt, vts, at):
                kv = kTt[:].rearrange("p (t c q) -> p t c q", t=NT, c=ND)
                qv = qTt[:].rearrange("p (t c q) -> p t c q", t=NT, c=ND)
                r0 = (h % 2) * 64
                for nch in range(2):
                    p_q = [sb.tile([128, 4, 512], BF16,
                                   name=f"p_s{s}h{h}n{nch}q{i}", tag="p", bufs=3)
                           for i in range(2)]
                    for mp in range(NT // 2):
                        st = ps.tile([128, 1024], F32, tag="st", bufs=2,
                                     name=f"st_{s}_{h}_{nch}_{mp}")
                        for u in range(2):
                            nc.tensor.matmul(
                                st[:, u * 512:(u + 1) * 512],
                                kv[r0:r0 + 64, mp * 2 + u, h // 2],
                                qv[r0:r0 + 64, nch * 4:(nch + 1) * 4, h // 2],
                            )
                        nc.scalar.activation(
                            p_q[mp // 2][:, (mp % 2) * 2:(mp % 2) * 2 + 2, :],
                            st[:], EXP, scale=float(SCALE))
                    pv = ps.tile([128, 260], F32, name=f"pv_{s}_{h}_{nch}",
                                 tag="pv", bufs=2)
                    for ql in range(4):
                        for m in range(NT):
                            nc.tensor.matmul(
                                pv[:, ql * 65:ql * 65 + 65],
                                p_q[m // 4][:, m % 4, ql * 128:(ql + 1) * 128],
                                vts[m][:].rearrange("p (h c) -> p h c",
                                                    c=HD + 1)[:, h],
                                start=(m == 0), stop=(m == NT - 1))
                    pvv = pv[:].rearrange("p (q c) -> p q c", c=65)
                    rc = sb.tile([128, 4], F32, name=f"rc_{s}_{h}_{nch}",
                                 tag="rc", bufs=2)
                    nc.vector.reciprocal_approx_fast(rc[:], pvv[:, :, 64])
                    av = at[:].rearrange("p (q h c) -> p q h c", q=NT, h=NH)
                    nc.vector.tensor_tensor(
                        av[:, nch * 4:(nch + 1) * 4, h, :],
                        pvv[:, :, 0:64], _bcast_last(rc[:], HD), mul)

            # ---------------- output projection ----------------
            def proj_out(s, qt, at):
                atq = sb.tile([128, ND, 128], BF16, name=f"atT_{s}_{qt}",
                              tag="attnT", bufs=2)
                nc.sync.dma_start_transpose(
                    atq[:], at[:, qt * 1024:(qt + 1) * 1024])
                for half in range(2):
                    py = ps.tile([128, 512], F32, name=f"yp_{s}_{qt}_{half}",
                                 tag="mm512", bufs=2)
                    for d in range(ND):
                        nc.tensor.matmul(
                            py[:],
                            atq[:, d, :],
                            wpr[:, d, half * 512:(half + 1) * 512],
                            start=(d == 0), stop=False)
                    nc.tensor.matmul(
                        py[:],
                        ones_r[:], bproj[:, half * 512:(half + 1) * 512],
                        start=False, stop=True)
                    ysb = sb.tile([128, 512], BF16, name=f"y_{s}_{qt}_{half}",
                                  tag="ysb", bufs=2)
                    nc.vector.tensor_copy(ysb[:], py[:])
                    nc.sync.dma_start(
                        y_d.ap()[s, qt * 128:(qt + 1) * 128,
                                 half * 512:(half + 1) * 512],
                        ysb[:])

            # ================= emission schedule =================
            qT = [sb.tile([128, NT * ND * 128], BF16, name=f"qT_s{s}", tag="qT",
                          bufs=2) for s in range(BPC)]
            kT = [sb.tile([128, NT * ND * 128], BF16, name=f"kT_s{s}", tag="kT",
                          bufs=2) for s in range(BPC)]
            attn = [sb.tile([128, NT * 1024], BF16, name=f"attn_s{s}", tag="attn",
                            bufs=2) for s in range(BPC)]
            vsb = [[sb.tile([128, NH * (HD + 1)], BF16, name=f"v_s{s}_{t}",
                            tag=f"v{t}", bufs=2) for t in range(NT)]
                   for s in range(BPC)]

            def slab_items(s, sect, half, xT):
                """fetch + the 8 per-t work items for one weight slab."""
                items = [("w", (s, sect, half))]
                for t in range(NT):
                    if sect == 2:
                        items.append(("v", (s, half, t, xT)))
                    else:
                        items.append(("qk", (s, sect, half, t, xT)))
                return items

            def run_item(it, weng=None):
                kind, args = it
                if kind == "w":
                    fetch_w(*args, eng=weng)
                elif kind == "v":
                    s_, half, t, xT = args
                    v_half(s_, half, t, xT, vsb[s_][t])
                else:
                    s_, sect, half, t, xT = args
                    qk_half(s_, sect, half, t, xT,
                            qT[s_] if sect == 0 else kT[s_])

            def run_slabs(slabs):
                """emit slab work with fetches hoisted 2 slabs ahead."""
                items = [slab_items(s_, sect, half, xT)
                         for (s_, sect, half, xT) in slabs]
                # reorder: fetch of slab i+2 goes before slab i's t-work
                out = []
                fetched = 0
                for i in range(len(items)):
                    while fetched <= min(i + 2, len(items) - 1):
                        out.append(items[fetched][0]); fetched += 1
                    out.extend(items[i][1:])
                return out

            # ---- phase P(s0): V + K/Q half0 of sample 0 (heads 0-7
            # of s0 only need these; half1 folds into A(s0)) ----
            xT0 = xT_tiles(0)
            for it in run_slabs([(0, 2, 0, xT0), (0, 2, 1, xT0),
                                 (0, 1, 0, xT0), (0, 0, 0, xT0)]):
                run_item(it, weng=nc.scalar)

            # ---- phase A(s0): s0 attention; heads 0-7 absorb s0 K/Q half1
            # + start of s1 projection; heads 8-15 absorb the rest ----
            xT1 = sb.tile([128, ND, N], BF16, name="xT_s1", tag="xT")
            s1_filler = (run_slabs([(0, 1, 1, xT0), (0, 0, 1, xT0)])
                         + run_slabs([(1, 2, 0, xT1), (1, 2, 1, xT1),
                                      (1, 1, 0, xT1), (1, 0, 0, xT1)]))
            run_item(s1_filler[0])
            run_item(s1_filler[1])
            nc.sync.dma_start(
                xT1[:], xT_d.ap()[1].rearrange("(c p) n -> p c n", p=128))
            fi = 2
            for h in range(NH):
                attention_head(0, h, qT[0], kT[0], vsb[0], attn[0])
                take = (len(s1_filler) * (h + 1)) // NH - fi
                for _ in range(take):
                    run_item(s1_filler[fi]); fi += 1

            # ---- phase A(s1): s1 attention (heads 0-7 ready) + s1 K/Q
            # half1 projection + out-proj of s0 on the late heads ----
            late = run_slabs([(1, 1, 1, xT1), (1, 0, 1, xT1)])
            li = 0
            for h in range(NH):
                attention_head(1, h, qT[1], kT[1], vsb[1], attn[1])
                if h < 6:
                    take = (len(late) * (h + 1)) // 6 - li
                    for _ in range(take):
                        run_item(late[li]); li += 1
                if 7 <= h < 15:
                    proj_out(0, h - 7, attn[0])

            # ---- out-proj of s1 ----
            for qt in range(NT):
                proj_out(1, qt, attn[1])

    nc.compile()
    return nc


_NC_CACHE = None


def kernel(x, w_qkv, w_proj, b_proj):
    global _NC_CACHE, last_exec_time_ns
    import ml_dtypes

    x = np.asarray(x, np.float32)
    w_qkv = np.asarray(w_qkv, np.float32)
    w_proj = np.asarray(w_proj, np.float32)
    b_proj = np.asarray(b_proj, np.float32)

    if _NC_CACHE is None:
        _NC_CACHE = _build()
    nc = _NC_CACHE

    cosn, sinn = _freq_tables()
    cosn = cosn.astype(ml_dtypes.bfloat16)
    sinn = sinn.astype(ml_dtypes.bfloat16)
    wqkvT = np.ascontiguousarray(w_qkv.T).astype(ml_dtypes.bfloat16)
    wprojT = np.ascontiguousarray(w_proj.T).astype(ml_dtypes.bfloat16)
    bproj16 = b_proj.reshape(1, DIM).astype(ml_dtypes.bfloat16)
    ones16 = np.ones((1, 128), ml_dtypes.bfloat16)

    in_maps = []
    for c in range(NCORES):
        xs = x[c * BPC:(c + 1) * BPC]                       # [2, N, DIM]
        xT = np.ascontiguousarray(xs.transpose(0, 2, 1)).astype(ml_dtypes.bfloat16)
        in_maps.append({
            "xT": xT, "wqkvT": wqkvT, "wprojT": wprojT,
            "bproj": bproj16, "ones": ones16, "cosn": cosn, "sinn": sinn,
        })

    trace = bool(os.environ.get("KERNEL_TRACE"))
    kwargs = {}
    if trace:
        kwargs["trace"] = True
        td = os.environ.get("KERNEL_TRACE_DIR")
        if td:
            kwargs["tmpdir"] = td
    res = bass_utils.run_bass_kernel_spmd(
        nc, in_maps, core_ids=list(range(NCORES)), **kwargs)
    last_exec_time_ns = res.exec_time_ns
    out = np.concatenate([np.asarray(res.results[c]["y"]) for c in range(NCORES)],
                         axis=0)
    return np.ascontiguousarray(out.reshape(B, N, DIM).astype(np.float32))


if __name__ == "__main__":
    rng = np.random.default_rng(0)
    xs = rng.standard_normal((B, N, DIM), dtype=np.float32)
    wq = rng.standard_normal((3 * DIM, DIM), dtype=np.float32) / 32
    wp = rng.standard_normal((DIM, DIM), dtype=np.float32) / 32
    bp = np.zeros(DIM, np.float32)
    y = kernel(xs, wq, wp, bp)
    print("y", y.shape, y.dtype, float(np.abs(y).max()))
